# revision 13
# baseline (speedup 1.0000x reference)
"""Distributed 2-layer GCN on 8 TRN2 NeuronCores (Bass/Tile).

Reference computation (PyG-style GCNConv, f32):
    e  = embed_table[node_tokens]            # [N, 256]
    x0 = e @ Wn^T + bn                       # [N, 128]
    h1 = Ahat @ (x0 @ w1^T) + b1 ; z1 = relu(h1)
    h2 = Ahat @ (z1 @ w2^T) + b2             # output [N, 128]
  with Ahat = D^-1/2 (A + I) D^-1/2, deg from dst(+self loops).

Sharding: nodes are partitioned contiguously across the 8 cores (6250 each,
padded to 6272 = 49 tiles of 128). Each core aggregates the edges pointing
at its own nodes, projects, and writes its output shard.

v7 design:
  - The embedding lookup + input projection is folded on the host into a
    projected table Tp = embed_table @ Wn^T + bn  [V, 128] bf16 (parameter-
    only preprocessing, stored as lo/hi halves for int16 gather indexing).
  - LAYER 1 NEEDS NO COLLECTIVE AT ALL: its messages are gathered straight
    from the replicated Tp (idx = tok[src]); the per-src norm dinv[src] is
    applied per chunk on the (otherwise idle) vector engine via a
    per-partition tensor_scalar multiply (slot == partition). Self loops
    are ordinary slots (idx = tok[i], scale dinv[i]). Layer 1 therefore
    starts at t~0, fully overlapping the fabric's first-collective barrier
    (~150-250us), which a dummy AllGather kicks off immediately.
  - z1 is sharded into TWO pieces per core: piece A = tiles 0..23, piece B
    = tiles 24..48; each piece is AllGathered as soon as its rows close
    (AG_a overlaps layer 1's tail, AG_b overlaps layer 2's piece-A phase).
    The 8*3072 / 8*3200-row gathered pieces each fit int16 index space.
  - ONE shared slot/E-matrix layout serves both layers: edge slots are
    bucketed per (src-node piece, dst-tile group of 4), with each bucket
    split into (src-token half) sections chunk-aligned so every 128-slot
    chunk has a single gather source in BOTH address spaces (Tp halves for
    layer 1, z1 pieces for layer 2). Slots are sorted by (half, dst tile,
    src) and deduped per (dst tile, src); E blocks are 0/1 multiplicity
    counts, exact in fp8, resident in SBUF, reused by both layers. Layer 2
    gathers the same slots from z1 (self slots read the core's own rows,
    already dinv-scaled, so no vector multiply).
  - Layer 1 processes buckets in (group: piece A then B) pair order,
    accumulating each 4-tile PSUM bank across both buckets. Layer 2
    processes all piece-A buckets first (so they only wait on AG_a),
    spilling each bank to an SBUF partial, then reopens from the partial
    during the piece-B phase.
  - Per dst tile close: agg[dst,feat] -> copy*dinv[dst] -> transpose ->
    w^T matmul -> bias(+relu) -> transpose -> store copy (*dinv for z1's
    pre-scale; plain f32 for the final output) -> one DMA per group ->
    piece AllGather dispatch after groups 5 / 12.
"""

import os

import numpy as np

import concourse.bacc as bacc
from bass_rust import InstructionNameOrderedSet
import concourse.mybir as mybir
import concourse.tile as tile
from concourse.bass_utils import run_bass_kernel_spmd
from concourse.library_config import mlp

# Problem shape (hardcoded per harness contract)
N = 50000
E = 600000
V = 50000
D_IN = 256
D = 128
NCORES = 8

NPC = N // NCORES            # 6250 nodes per core
TPC = (NPC + 127) // 128     # 49 tiles per core
NPAD = TPC * 128             # 6272 padded nodes per core
PA_T = 24                    # piece A: tiles 0..23
PB_T = TPC - PA_T            # piece B: tiles 24..48
RA = PA_T * 128              # 3072 rows per core in piece A
RB = PB_T * 128              # 3200 rows per core in piece B
VLO = V // 2                 # 25000: projected-table split
GRP = 4                      # dst tiles per aggregation group (PSUM bank)
NGRP = (TPC + GRP - 1) // GRP
GMAXC = 8                    # max chunks (x128 slots) per dma_gather
NQ = 4                       # SWDGE queues
F32 = mybir.dt.float32
BF16 = mybir.dt.bfloat16
FP8 = mybir.dt.float8e4
I16 = mybir.dt.int16
MSGB = int(os.environ.get("KMSGB", "12"))   # msgs pool bufs


def _wrap_idx(idx_linear):
    """[n] -> [128, n/16] int16: position j at [j%16, j//16], replicated x8."""
    n = idx_linear.shape[0]
    assert n % 16 == 0
    w = idx_linear.astype(np.int16).reshape(-1, 16).T
    return np.tile(w, (8, 1))


def _preprocess(node_tokens, edge_index):
    """Build per-core host arrays + the (core-uniform) schedules.

    Shared slot layout: buckets (piece p, group g), each split into two
    chunk-aligned sections by src-token half h. Within a section, slots
    sorted by (dst tile, src), deduped per (dst tile, src). Self edges
    (i -> i) are included for every padded node.

    Returns per-core {gidx1, gidx2, emat, dinvslot, dinv} plus layout:
      blocks1/blocks2: [(gidx_col_off, n_chunks, src_id, chunk_base)]
        src_id: layer 1 -> 0=tp_lo 1=tp_hi; layer 2 -> 0=z_fa 1=z_fb
      ops1/ops2: per linear chunk, ordered ops:
        ("bank", g) | ("reopen", g) | ("mm", t, eb, start, stop) |
        ("pclose", g) | ("close", t, g)
      n_eb, nch: E-block and chunk counts
    """
    import ml_dtypes

    src_e = np.asarray(edge_index[0], dtype=np.int64)
    dst_e = np.asarray(edge_index[1], dtype=np.int64)
    tok = np.asarray(node_tokens, dtype=np.int64)

    deg = (np.bincount(dst_e, minlength=N) + 1).astype(np.float32)
    dinv = (1.0 / np.sqrt(deg)).astype(np.float32)
    # per padded-node-id token / dinv (pads: tok 0, dinv 0)
    tokp = np.zeros(NCORES * NPAD, np.int64)
    dinvp = np.zeros(NCORES * NPAD, np.float32)
    for c in range(NCORES):
        tokp[c * NPAD : c * NPAD + NPC] = tok[c * NPC : (c + 1) * NPC]
        dinvp[c * NPAD : c * NPAD + NPC] = dinv[c * NPC : (c + 1) * NPC]

    # edge list in padded-node-id space + self loops for every padded node
    pid = lambda n: (n // NPC) * NPAD + (n % NPC)
    allid = np.arange(NCORES * NPAD)
    src = np.concatenate([pid(src_e), allid])
    dst = np.concatenate([pid(dst_e), allid])

    core = dst // NPAD
    dloc = dst % NPAD
    tloc = dloc // 128
    dcol = (dloc % 128).astype(np.int64)
    sloc = src % NPAD
    piece = (sloc >= RA).astype(np.int64)
    half = (tokp[src] >= VLO).astype(np.int64)
    grp = tloc // GRP

    # sort per (dst core, piece, group, half, dst tile, src)
    key = ((((core * 2 + piece) * NGRP + grp) * 2 + half) * TPC + tloc)
    order = np.lexsort((src, key))
    key_s = key[order]
    src_s = src[order]
    dcol_s = dcol[order]
    nkey = NCORES * 2 * NGRP * 2 * TPC
    counts_raw = np.bincount(key_s, minlength=nkey)
    starts = np.zeros(nkey + 1, dtype=np.int64)
    np.cumsum(counts_raw, out=starts[1:])

    def kid(c, p, g, h, t):
        return (((c * 2 + p) * NGRP + g) * 2 + h) * TPC + t

    groups = [list(range(g * GRP, min((g + 1) * GRP, TPC)))
              for g in range(NGRP)]

    # no dedup: every edge (incl. duplicates) is its own slot, so every E
    # row is pure one-hot and E can be generated on-device from dcol values
    counts = {}
    for c in range(NCORES):
        for p in (0, 1):
            for g, ts in enumerate(groups):
                for h in (0, 1):
                    for t in ts:
                        counts[(c, p, g, h, t)] = int(
                            counts_raw[kid(c, p, g, h, t)])

    # section chunk counts (maxed over cores) and per-core slot offsets
    nsec = np.zeros((2, NGRP, 2), np.int64)
    tstart = {}
    for p in (0, 1):
        for g, ts in enumerate(groups):
            for h in (0, 1):
                mx = 0
                for c in range(NCORES):
                    acc = 0
                    for t in ts:
                        tstart[(c, p, g, h, t)] = acc
                        acc += counts[(c, p, g, h, t)]
                    mx = max(mx, acc)
                nsec[p, g, h] = -(-mx // 128)
                assert nsec[p, g, h] >= 1

    # linear chunk ids: buckets (p, g) in p-major order; sections h0, h1
    bucket_base = {}
    sec_base = {}
    nch = 0
    for p in (0, 1):
        for g in range(NGRP):
            bucket_base[(p, g)] = nch
            sec_base[(p, g, 0)] = nch
            sec_base[(p, g, 1)] = nch + int(nsec[p, g, 0])
            nch += int(nsec[p, g, 0] + nsec[p, g, 1])

    # union chunk span (bucket-relative -> absolute) per (p, g, h, t)
    span = {}
    for p in (0, 1):
        for g, ts in enumerate(groups):
            for h in (0, 1):
                b = sec_base[(p, g, h)]
                for t in ts:
                    k0 = min(tstart[(c, p, g, h, t)] // 128
                             for c in range(NCORES))
                    k1 = max(-(-(tstart[(c, p, g, h, t)]
                                 + counts[(c, p, g, h, t)]) // 128)
                             for c in range(NCORES))
                    span[(p, g, h, t)] = (b + k0, b + max(k1, k0))

    # E-block ids in (p, g, h-section, chunk, tile) emission order
    eb_of = {}
    n_eb = 0
    bucket_mms = {}
    for p in (0, 1):
        for g, ts in enumerate(groups):
            mms = []
            for h in (0, 1):
                b = sec_base[(p, g, h)]
                for k in range(b, b + int(nsec[p, g, h])):
                    for t in ts:
                        k0, k1 = span[(p, g, h, t)]
                        if k0 <= k < k1:
                            eb_of[(k, t)] = n_eb
                            mms.append((k, t, n_eb))
                            n_eb += 1
            assert mms, f"empty bucket p={p} g={g}"
            bucket_mms[(p, g)] = mms
    ebarr = np.full((nch, GRP), -1, np.int64)
    for (k, t), eb in eb_of.items():
        ebarr[k, t % GRP] = eb

    def tile_close_chunk(g, t):
        """Chunk at which tile t closes (its last mm in bucket (1, g))."""
        cands = [k for (k, tt, _) in bucket_mms[(1, g)] if tt == t]
        return cands[-1] if cands else bucket_mms[(1, g)][-1][0]

    # ---- layer 1 ops: bucket pair order (0,g),(1,g); accumulate across ----
    ops1 = [[] for _ in range(nch)]
    for g in range(NGRP):
        mms0, mms1 = bucket_mms[(0, g)], bucket_mms[(1, g)]
        ops1[bucket_base[(0, g)]].append(("bank", g))
        for i, (k, t, eb) in enumerate(mms0):
            ops1[k].append(("mm", t, eb, i == 0, False))
        for i, (k, t, eb) in enumerate(mms1):
            ops1[k].append(("mm", t, eb, False, i == len(mms1) - 1))
        for t in groups[g]:
            ops1[tile_close_chunk(g, t)].append(("close", t, g))

    # ---- layer 2 ops: phase order; spill/reload SBUF partials ----
    ops2 = [[] for _ in range(nch)]
    for g in range(NGRP):
        mms0, mms1 = bucket_mms[(0, g)], bucket_mms[(1, g)]
        ops2[bucket_base[(0, g)]].append(("bank", g))
        for i, (k, t, eb) in enumerate(mms0):
            ops2[k].append(("mm", t, eb, i == 0, i == len(mms0) - 1))
        ops2[mms0[-1][0]].append(("pclose", g))
        ops2[bucket_base[(1, g)]].append(("reopen", g))
        for i, (k, t, eb) in enumerate(mms1):
            ops2[k].append(("mm", t, eb, False, i == len(mms1) - 1))
        for t in groups[g]:
            ops2[tile_close_chunk(g, t)].append(("close", t, g))

    kind_rank = {"bank": 0, "reopen": 0, "mm": 1, "pclose": 2, "close": 2}
    for ops in (ops1, ops2):
        for k in range(nch):
            ops[k].sort(key=lambda op: (kind_rank[op[0]],
                                        op[1] if op[0] == "close" else -1))

    # ---- gather blocks ----
    # layer 1: bucket PAIR order (0,g),(1,g) to match ops1's cross-bucket
    # bank accumulation; blocks per (p, g, h) section (source = Tp half h)
    blocks1 = []
    for g in range(NGRP):
        for p in (0, 1):
            for h in (0, 1):
                b = sec_base[(p, g, h)]
                nk = int(nsec[p, g, h])
                off = 0
                while off < nk:
                    n = min(GMAXC, nk - off)
                    blocks1.append([None, n, h, b + off])
                    off += n
    # layer 2: per (p, g) bucket (source = z1 piece p)
    blocks2 = []
    for p in (0, 1):
        for g in range(NGRP):
            b = bucket_base[(p, g)]
            nk = int(nsec[p, g, 0] + nsec[p, g, 1])
            off = 0
            while off < nk:
                n = min(GMAXC, nk - off)
                blocks2.append([None, n, p, b + off])
                off += n
    col = 0
    for blk in blocks1:
        blk[0] = col
        col += blk[1] * 8
    g1cols = col
    col = 0
    for blk in blocks2:
        blk[0] = col
        col += blk[1] * 8
    g2cols = col

    per_core = []
    for c in range(NCORES):
        # linear slot tables (chunk-id space)
        lin1 = np.zeros(nch * 128, np.int64)         # Tp half-space index
        lin2 = np.zeros(nch * 128, np.int64)         # z1 piece-space index
        dslot = np.zeros(nch * 128, np.float32)      # dinv[src] per slot
        dcole = np.full((128, n_eb), -1.0, np.float32)
        for p in (0, 1):
            for g, ts in enumerate(groups):
                for h in (0, 1):
                    b = sec_base[(p, g, h)]
                    for t in ts:
                        k = kid(c, p, g, h, t)
                        s0e, ne = int(starts[k]), counts[(c, p, g, h, t)]
                        if ne == 0:
                            continue
                        uidx = src_s[s0e : s0e + ne]
                        dcols = dcol_s[s0e : s0e + ne]
                        s0 = b * 128 + tstart[(c, p, g, h, t)]
                        lin1[s0 : s0 + ne] = tokp[uidx] - h * VLO
                        sl = uidx % NPAD
                        lin2[s0 : s0 + ne] = np.where(
                            sl < RA, (uidx // NPAD) * RA + sl,
                            (uidx // NPAD) * RB + (sl - RA))
                        dslot[s0 : s0 + ne] = dinvp[uidx]
                        gslots = (s0 - b * 128) + np.arange(ne)
                        ebs = ebarr[b + gslots // 128, t % GRP]
                        assert (ebs >= 0).all()
                        dcole[gslots % 128, ebs] = dcols
        gidx1 = np.concatenate(
            [_wrap_idx(lin1[b * 128 : (b + n) * 128])
             for (_, n, _, b) in blocks1], axis=1)
        gidx2 = np.concatenate(
            [_wrap_idx(lin2[b * 128 : (b + n) * 128])
             for (_, n, _, b) in blocks2], axis=1)
        dinvslot = np.ascontiguousarray(
            dslot.reshape(nch, 128).T)               # [128, nch]

        dv = dinvp[c * NPAD : (c + 1) * NPAD]
        dinv_loc = np.ascontiguousarray(dv.reshape(TPC, 128).T)

        per_core.append({"gidx1": gidx1, "gidx2": gidx2, "dcole": dcole,
                         "dinvslot": dinvslot, "dinv": dinv_loc})

    layout = {"blocks1": blocks1, "blocks2": blocks2, "ops1": ops1,
              "ops2": ops2, "n_eb": n_eb, "nch": nch,
              "g1cols": g1cols, "g2cols": g2cols}
    return per_core, layout


def _build(layout):
    blocks1 = layout["blocks1"]
    blocks2 = layout["blocks2"]
    ops1 = layout["ops1"]
    ops2 = layout["ops2"]
    n_eb = layout["n_eb"]
    nch = layout["nch"]

    nc = bacc.Bacc("TRN2", target_bir_lowering=False, debug=False,
                   num_devices=NCORES, num_swdge_queues=NQ)

    tp_lo = nc.dram_tensor("tp_lo", [VLO, D], BF16, kind="ExternalInput")
    tp_hi = nc.dram_tensor("tp_hi", [V - VLO, D], BF16, kind="ExternalInput")
    g1_d = nc.dram_tensor("gidx1", [128, layout["g1cols"]], I16,
                          kind="ExternalInput")
    g2_d = nc.dram_tensor("gidx2", [128, layout["g2cols"]], I16,
                          kind="ExternalInput")
    dcole_d = nc.dram_tensor("dcole", [128, n_eb], F32,
                             kind="ExternalInput")
    iota_d = nc.dram_tensor("iota", [128, 128], F32, kind="ExternalInput")
    dslot_d = nc.dram_tensor("dinvslot", [128, nch], F32,
                             kind="ExternalInput")
    dinv_d = nc.dram_tensor("dinv", [128, TPC], F32, kind="ExternalInput")
    w1t_d = nc.dram_tensor("w1t", [128, D], BF16, kind="ExternalInput")
    w2t_d = nc.dram_tensor("w2t", [128, D], BF16, kind="ExternalInput")
    bias_d = nc.dram_tensor("bias", [128, 2], F32, kind="ExternalInput")
    identb_d = nc.dram_tensor("identb", [128, 128], BF16, kind="ExternalInput")
    ident8_d = nc.dram_tensor("ident8", [128, 128], FP8, kind="ExternalInput")
    out_d = nc.dram_tensor("out", [NPAD, D], F32, kind="ExternalOutput")

    ACT = mybir.ActivationFunctionType

    with tile.TileContext(nc) as tc:
        with (
            tc.tile_pool(name="const", bufs=1) as cp,
            tc.tile_pool(name="msgs", bufs=MSGB) as msgp,
            tc.tile_pool(name="part", bufs=NGRP) as partp,
            tc.tile_pool(name="work", bufs=3) as wk,
            tc.tile_pool(name="stage", bufs=3) as stg,
            tc.tile_pool(name="psG", bufs=3, space="PSUM") as psG,
            tc.tile_pool(name="psT", bufs=2, space="PSUM") as psT,
            tc.tile_pool(name="psB", bufs=2, space="PSUM") as psB,
            tc.tile_pool(name="psC", bufs=1, space="PSUM") as psC,
            tc.tile_pool(name="dram", bufs=1, space="DRAM") as dram,
        ):
            nc.gpsimd.load_library(mlp)

            z1_a = dram.tile([RA, D], BF16)
            z1_b = dram.tile([RB, D], BF16)
            z1_fa = dram.tile([NCORES * RA, D], BF16, addr_space="Shared")
            z1_fb = dram.tile([NCORES * RB, D], BF16, addr_space="Shared")
            dum_l = dram.tile([16, D], BF16)
            dum_f = dram.tile([NCORES * 16, D], BF16, addr_space="Shared")

            def collective(z_loc, z_full):
                return nc.gpsimd.collective_compute(
                    "AllGather", mybir.AluOpType.bypass,
                    replica_groups=[list(range(NCORES))],
                    ins=[z_loc.opt()], outs=[z_full.opt()])

            # Warm up the collective fabric (first-collective barrier takes
            # 150-250us; run it concurrently with layer 1 from t=0).
            collective(dum_l, dum_f)

            g1_sb = cp.tile([128, layout["g1cols"]], I16)
            g2_sb = cp.tile([128, layout["g2cols"]], I16)
            emat_sb = cp.tile([128, n_eb, 128], FP8)
            dcole_sb = cp.tile([128, n_eb], F32)
            iota_sb = cp.tile([128, 128], F32)
            dslot_sb = cp.tile([128, nch], F32)
            dinv_sb = cp.tile([128, TPC], F32)
            w1t_sb = cp.tile([128, D], BF16)
            w2t_sb = cp.tile([128, D], BF16)
            bias_sb = cp.tile([128, 2], F32)
            identb_sb = cp.tile([128, 128], BF16)
            ident8_sb = cp.tile([128, 128], FP8)
            gq = [0, layout["g1cols"] // 4, layout["g1cols"] // 2,
                  3 * layout["g1cols"] // 4, layout["g1cols"]]
            for qi in range(4):
                nc.sync.dma_start(g1_sb[:, gq[qi] : gq[qi + 1]],
                                  g1_d[:, gq[qi] : gq[qi + 1]])
            nc.sync.dma_start(dslot_sb[:], dslot_d[:])
            nc.sync.dma_start(dcole_sb[:], dcole_d[:])
            nc.sync.dma_start(iota_sb[:], iota_d[:])
            # Generate the one-hot E blocks on the vector engine (compare
            # dcol against an iota row): saves a 14+MB constant load on the
            # DMA engines, which are the end-to-end bottleneck.
            EBB = 32
            for a in range(0, n_eb, EBB):
                b = min(a + EBB, n_eb)
                nc.vector.tensor_tensor(
                    emat_sb[:, a:b, :],
                    dcole_sb[:, a:b].to_broadcast((128, b - a, 128)),
                    iota_sb[:].unsqueeze(1).to_broadcast((128, b - a, 128)),
                    mybir.AluOpType.is_equal)
            nc.sync.dma_start(dinv_sb[:], dinv_d[:])
            nc.sync.dma_start(w1t_sb[:], w1t_d[:])
            nc.sync.dma_start(w2t_sb[:], w2t_d[:])
            nc.sync.dma_start(bias_sb[:], bias_d[:])
            nc.sync.dma_start(identb_sb[:], identb_d[:])
            nc.sync.dma_start(ident8_sb[:], ident8_d[:])
            nc.sync.dma_start(g2_sb[:], g2_d[:])

            qn = [0]

            def next_q():
                qn[0] = (qn[0] + 1) % NQ
                return qn[0]

            # Pre-touch the msgs buffers so pad slots never multiply
            # uninitialized SBUF into the PSUM accumulation.
            for _ in range(MSGB):
                mz = msgp.tile([128, GMAXC, D], BF16, name="m", tag="m")
                nc.vector.memset(mz[:], 0)

            def run_layer(blocks, ops, srcs, use_scale, wt_sb, bias_col,
                          relu, dest_a, dest_b, out_colls):
                open_ps = {}
                grp_ps = {}
                grp_part = {}
                grp_stage = {}
                grp_closed = {}

                def ntile_of(g):
                    return min(GRP, TPC - g * GRP)

                def op_bank(g, moving):
                    ntile = ntile_of(g)
                    ps = psG.tile([128, GRP * 128], F32, name="agg", tag="pG")
                    grp_ps[g] = ps
                    if moving is not None:
                        nc.tensor.matmul(
                            ps[:, 0 : ntile * 128], ident8_sb[:], moving,
                            start=True, stop=False, skip_group_check=True)
                    for j in range(ntile):
                        open_ps[g * GRP + j] = ps[:, j * 128 : (j + 1) * 128]
                    grp_stage[g] = stg.tile(
                        [128, ntile, D], F32 if dest_b is None else BF16,
                        name="stage1", tag="st1")
                    grp_closed[g] = 0

                def op_pclose(g):
                    ntile = ntile_of(g)
                    part = partp.tile([128, GRP, D], BF16, name="part",
                                      tag="part")
                    grp_part[g] = part
                    ps = grp_ps.pop(g)
                    nc.scalar.activation(
                        part[:, 0:ntile, :].rearrange("p t f -> p (t f)"),
                        ps[:, 0 : ntile * 128], ACT.Copy)
                    for j in range(ntile):
                        del open_ps[g * GRP + j]
                    del grp_stage[g], grp_closed[g]

                def op_reopen(g):
                    ntile = ntile_of(g)
                    part = grp_part.pop(g)
                    op_bank(g, part[:, 0:ntile, :]
                            .rearrange("p t f -> p (t f)"))

                def op_close(t, g):
                    ntile = ntile_of(g)
                    agg_sb = wk.tile([128, 128], BF16, name="agg_sb",
                                     tag="agg_sb")
                    nc.scalar.activation(agg_sb[:], open_ps.pop(t), ACT.Copy,
                                         scale=dinv_sb[:, t : t + 1])
                    aggT_ps = psT.tile([128, 128], BF16, name="aggT",
                                       tag="pT")
                    nc.tensor.matmul(aggT_ps[:], agg_sb[:], identb_sb[:],
                                     is_transpose=True, start=True, stop=True)
                    aggT_sb = wk.tile([128, 128], BF16, name="aggT_sb",
                                      tag="aggT_sb")
                    nc.scalar.activation(aggT_sb[:], aggT_ps[:], ACT.Copy)
                    yT_ps = psB.tile([128, 128], F32, name="yT", tag="pB")
                    nc.tensor.matmul(yT_ps[:], wt_sb[:], aggT_sb[:],
                                     start=True, stop=True)
                    yT_sb = wk.tile([128, 128], BF16, name="yT_sb",
                                    tag="yT_sb")
                    nc.scalar.activation(yT_sb[:], yT_ps[:],
                                         ACT.Relu if relu else ACT.Identity,
                                         bias=bias_col)
                    y_ps = psC.tile([128, 128], BF16, name="y", tag="pC")
                    nc.tensor.matmul(y_ps[:], yT_sb[:], identb_sb[:],
                                     is_transpose=True, start=True, stop=True)
                    gt0 = g * GRP
                    if dest_b is None:
                        nc.scalar.activation(grp_stage[g][:, t - gt0, :],
                                             y_ps[:], ACT.Copy)
                    else:
                        nc.scalar.activation(grp_stage[g][:, t - gt0, :],
                                             y_ps[:], ACT.Copy,
                                             scale=dinv_sb[:, t : t + 1])
                    grp_closed[g] += 1
                    if grp_closed[g] == ntile:
                        if dest_b is None:
                            dst_rows = dest_a[gt0 * 128
                                              : (gt0 + ntile) * 128, :]
                        elif gt0 < PA_T:
                            dst_rows = dest_a[gt0 * 128
                                              : (gt0 + ntile) * 128, :]
                        else:
                            dst_rows = dest_b[(gt0 - PA_T) * 128
                                              : (gt0 - PA_T + ntile) * 128, :]
                        nc.sync.dma_start(
                            dst_rows.rearrange("(t p) f -> p t f", p=128),
                            grp_stage[g][:])
                        del grp_ps[g], grp_stage[g], grp_closed[g]
                        if out_colls is not None:
                            if g == PA_T // GRP - 1:
                                out_colls[0]()
                            elif g == NGRP - 1:
                                out_colls[1]()

                gsb = g1_sb if use_scale else g2_sb
                for coloff, n, sid, base in blocks:
                    msgs = msgp.tile([128, GMAXC, D], BF16, name="m",
                                     tag="m")
                    nc.gpsimd.dma_gather(
                        msgs[:, 0:n, :], srcs[sid],
                        gsb[:, coloff : coloff + n * 8],
                        n * 128, n * 128, D, queue_num=next_q())
                    if use_scale:
                        # one broadcast multiply per gather block: slot ==
                        # partition, so dinv[src] is a per-partition scalar
                        # replicated along feats via a stride-0 AP
                        nc.vector.tensor_tensor(
                            msgs[:, 0:n, :], msgs[:, 0:n, :],
                            dslot_sb[:, base : base + n]
                            .to_broadcast((128, n, 128)),
                            mybir.AluOpType.mult)
                    for k in range(n):
                        ck = base + k
                        for op in ops[ck]:
                            if op[0] == "bank":
                                op_bank(op[1], None)
                            elif op[0] == "reopen":
                                op_reopen(op[1])
                            elif op[0] == "mm":
                                _, t, eb, st, sp = op
                                nc.tensor.matmul(
                                    open_ps[t], emat_sb[:, eb, :],
                                    msgs[:, k, :], start=st, stop=sp,
                                    skip_group_check=True)
                            elif op[0] == "pclose":
                                op_pclose(op[1])
                            elif op[0] == "close":
                                op_close(op[1], op[2])

            run_layer(blocks1, ops1, (tp_lo[:], tp_hi[:]), True, w1t_sb,
                      bias_sb[:, 0:1], True, z1_a, z1_b,
                      [lambda: collective(z1_a, z1_fa),
                       lambda: collective(z1_b, z1_fb)])
            run_layer(blocks2, ops2, (z1_fa[:], z1_fb[:]), False, w2t_sb,
                      bias_sb[:, 1:2], False, out_d.ap(), None, None)

    nc.compile()
    return nc


_CACHE = {}


def _run(inputs, trace=False):
    import ml_dtypes

    node_tokens = np.asarray(inputs["node_tokens"])
    edge_index = np.asarray(inputs["edge_index"])
    embed_table = np.asarray(inputs["embed_table"], dtype=np.float32)
    Wn = np.asarray(inputs["W_node_w"], dtype=np.float32)
    bn = np.asarray(inputs["W_node_b"], dtype=np.float32)
    w1 = np.asarray(inputs["w1"], dtype=np.float32)
    b1 = np.asarray(inputs["b1"], dtype=np.float32)
    w2 = np.asarray(inputs["w2"], dtype=np.float32)
    b2 = np.asarray(inputs["b2"], dtype=np.float32)

    per_core, layout = _preprocess(node_tokens, edge_index)

    if "nc" not in _CACHE:
        _CACHE["nc"] = _build(layout)
    nc = _CACHE["nc"]

    # Parameter-only preprocessing: fold the embedding projection.
    Tp = (embed_table @ Wn.T + bn).astype(ml_dtypes.bfloat16)   # [V, 128]
    tp_lo = Tp[:VLO]
    tp_hi = Tp[VLO:]
    bias = np.stack([b1, b2], axis=1).astype(np.float32)
    identb = np.eye(128, dtype=ml_dtypes.bfloat16)
    ident8 = np.eye(128, dtype=ml_dtypes.float8_e4m3)
    iota = np.broadcast_to(np.arange(128, dtype=np.float32), (128, 128))
    iota = np.ascontiguousarray(iota)

    in_maps = []
    for c in range(NCORES):
        in_maps.append({
            "tp_lo": tp_lo, "tp_hi": tp_hi,
            "gidx1": per_core[c]["gidx1"],
            "gidx2": per_core[c]["gidx2"],
            "dcole": per_core[c]["dcole"],
            "iota": iota,
            "dinvslot": per_core[c]["dinvslot"],
            "dinv": per_core[c]["dinv"],
            "w1t": w1.T.astype(ml_dtypes.bfloat16),
            "w2t": w2.T.astype(ml_dtypes.bfloat16),
            "bias": bias, "identb": identb, "ident8": ident8,
        })

    res = run_bass_kernel_spmd(nc, in_maps, core_ids=list(range(NCORES)),
                               trace=trace)
    out = np.concatenate([res.results[c]["out"][:NPC] for c in range(NCORES)],
                         axis=0)
    return out.astype(np.float32), res


def kernel(**inputs):
    out, _ = _run(inputs, trace=False)
    return out


# revision 14
# speedup vs baseline: 1.1366x; 1.1366x over previous
"""Distributed 2-layer GCN on 8 TRN2 NeuronCores (Bass/Tile).

Reference computation (PyG-style GCNConv, f32):
    e  = embed_table[node_tokens]            # [N, 256]
    x0 = e @ Wn^T + bn                       # [N, 128]
    h1 = Ahat @ (x0 @ w1^T) + b1 ; z1 = relu(h1)
    h2 = Ahat @ (z1 @ w2^T) + b2             # output [N, 128]
  with Ahat = D^-1/2 (A + I) D^-1/2, deg from dst(+self loops).

Sharding: nodes are partitioned contiguously across the 8 cores (6250 each,
padded to 6272 = 49 tiles of 128). Each core aggregates the edges pointing
at its own nodes, projects, and writes its output shard.

v7 design:
  - The embedding lookup + input projection is folded on the host into a
    projected table Tp = embed_table @ Wn^T + bn  [V, 128] bf16 (parameter-
    only preprocessing, stored as lo/hi halves for int16 gather indexing).
  - LAYER 1 NEEDS NO COLLECTIVE AT ALL: its messages are gathered straight
    from the replicated Tp (idx = tok[src]); the per-src norm dinv[src] is
    applied per chunk on the (otherwise idle) vector engine via a
    per-partition tensor_scalar multiply (slot == partition). Self loops
    are ordinary slots (idx = tok[i], scale dinv[i]). Layer 1 therefore
    starts at t~0, fully overlapping the fabric's first-collective barrier
    (~150-250us), which a dummy AllGather kicks off immediately.
  - z1 is sharded into TWO pieces per core: piece A = tiles 0..23, piece B
    = tiles 24..48; each piece is AllGathered as soon as its rows close
    (AG_a overlaps layer 1's tail, AG_b overlaps layer 2's piece-A phase).
    The 8*3072 / 8*3200-row gathered pieces each fit int16 index space.
  - ONE shared slot/E-matrix layout serves both layers: edge slots are
    bucketed per (src-node piece, dst-tile group of 4), with each bucket
    split into (src-token half) sections chunk-aligned so every 128-slot
    chunk has a single gather source in BOTH address spaces (Tp halves for
    layer 1, z1 pieces for layer 2). Slots are sorted by (half, dst tile,
    src) and deduped per (dst tile, src); E blocks are 0/1 multiplicity
    counts, exact in fp8, resident in SBUF, reused by both layers. Layer 2
    gathers the same slots from z1 (self slots read the core's own rows,
    already dinv-scaled, so no vector multiply).
  - Layer 1 processes buckets in (group: piece A then B) pair order,
    accumulating each 4-tile PSUM bank across both buckets. Layer 2
    processes all piece-A buckets first (so they only wait on AG_a),
    spilling each bank to an SBUF partial, then reopens from the partial
    during the piece-B phase.
  - Per dst tile close: agg[dst,feat] -> copy*dinv[dst] -> transpose ->
    w^T matmul -> bias(+relu) -> transpose -> store copy (*dinv for z1's
    pre-scale; plain f32 for the final output) -> one DMA per group ->
    piece AllGather dispatch after groups 5 / 12.
"""

import os

import numpy as np

import concourse.bacc as bacc
from bass_rust import InstructionNameOrderedSet
import concourse.mybir as mybir
import concourse.tile as tile
from concourse.bass_utils import run_bass_kernel_spmd
from concourse.library_config import mlp

# Problem shape (hardcoded per harness contract)
N = 50000
E = 600000
V = 50000
D_IN = 256
D = 128
NCORES = 8

NPC = N // NCORES            # 6250 nodes per core
TPC = (NPC + 127) // 128     # 49 tiles per core
NPAD = TPC * 128             # 6272 padded nodes per core
PA_T = 24                    # piece A: tiles 0..23
PB_T = TPC - PA_T            # piece B: tiles 24..48
RA = PA_T * 128              # 3072 rows per core in piece A
RB = PB_T * 128              # 3200 rows per core in piece B
VLO = V // 2                 # 25000: projected-table split
GRP = 4                      # dst tiles per aggregation group (PSUM bank)
NGRP = (TPC + GRP - 1) // GRP
GMAXC = 8                    # max chunks (x128 slots) per dma_gather
NQ = 4                       # SWDGE queues
F32 = mybir.dt.float32
BF16 = mybir.dt.bfloat16
FP8 = mybir.dt.float8e4
I16 = mybir.dt.int16
MSGB = int(os.environ.get("KMSGB", "12"))   # msgs pool bufs


def _wrap_idx(idx_linear):
    """[n] -> [128, n/16] int16: position j at [j%16, j//16], replicated x8."""
    n = idx_linear.shape[0]
    assert n % 16 == 0
    w = idx_linear.astype(np.int16).reshape(-1, 16).T
    return np.tile(w, (8, 1))


def _preprocess(node_tokens, edge_index):
    """Build per-core host arrays + the (core-uniform) schedules.

    Shared slot layout: buckets (piece p, group g), each split into two
    chunk-aligned sections by src-token half h. Within a section, slots
    sorted by (dst tile, src), deduped per (dst tile, src). Self edges
    (i -> i) are included for every padded node.

    Returns per-core {gidx1, gidx2, emat, dinvslot, dinv} plus layout:
      blocks1/blocks2: [(gidx_col_off, n_chunks, src_id, chunk_base)]
        src_id: layer 1 -> 0=tp_lo 1=tp_hi; layer 2 -> 0=z_fa 1=z_fb
      ops1/ops2: per linear chunk, ordered ops:
        ("bank", g) | ("reopen", g) | ("mm", t, eb, start, stop) |
        ("pclose", g) | ("close", t, g)
      n_eb, nch: E-block and chunk counts
    """
    import ml_dtypes

    src_e = np.asarray(edge_index[0], dtype=np.int64)
    dst_e = np.asarray(edge_index[1], dtype=np.int64)
    tok = np.asarray(node_tokens, dtype=np.int64)

    deg = (np.bincount(dst_e, minlength=N) + 1).astype(np.float32)
    dinv = (1.0 / np.sqrt(deg)).astype(np.float32)
    # per padded-node-id token / dinv (pads: tok 0, dinv 0)
    tokp = np.zeros(NCORES * NPAD, np.int64)
    dinvp = np.zeros(NCORES * NPAD, np.float32)
    for c in range(NCORES):
        tokp[c * NPAD : c * NPAD + NPC] = tok[c * NPC : (c + 1) * NPC]
        dinvp[c * NPAD : c * NPAD + NPC] = dinv[c * NPC : (c + 1) * NPC]

    # edge list in padded-node-id space + self loops for every padded node
    pid = lambda n: (n // NPC) * NPAD + (n % NPC)
    allid = np.arange(NCORES * NPAD)
    src = np.concatenate([pid(src_e), allid])
    dst = np.concatenate([pid(dst_e), allid])

    core = dst // NPAD
    dloc = dst % NPAD
    tloc = dloc // 128
    dcol = (dloc % 128).astype(np.int64)
    sloc = src % NPAD
    piece = (sloc >= RA).astype(np.int64)
    half = (tokp[src] >= VLO).astype(np.int64)
    grp = tloc // GRP

    # sort per (dst core, piece, group, half, dst tile, src)
    key = ((((core * 2 + piece) * NGRP + grp) * 2 + half) * TPC + tloc)
    order = np.lexsort((src, key))
    key_s = key[order]
    src_s = src[order]
    dcol_s = dcol[order]
    nkey = NCORES * 2 * NGRP * 2 * TPC
    counts_raw = np.bincount(key_s, minlength=nkey)
    starts = np.zeros(nkey + 1, dtype=np.int64)
    np.cumsum(counts_raw, out=starts[1:])

    def kid(c, p, g, h, t):
        return (((c * 2 + p) * NGRP + g) * 2 + h) * TPC + t

    groups = [list(range(g * GRP, min((g + 1) * GRP, TPC)))
              for g in range(NGRP)]

    # no dedup: every edge (incl. duplicates) is its own slot, so every E
    # row is pure one-hot and E can be generated on-device from dcol values
    counts = {}
    for c in range(NCORES):
        for p in (0, 1):
            for g, ts in enumerate(groups):
                for h in (0, 1):
                    for t in ts:
                        counts[(c, p, g, h, t)] = int(
                            counts_raw[kid(c, p, g, h, t)])

    # section chunk counts (maxed over cores) and per-core slot offsets
    nsec = np.zeros((2, NGRP, 2), np.int64)
    tstart = {}
    for p in (0, 1):
        for g, ts in enumerate(groups):
            for h in (0, 1):
                mx = 0
                for c in range(NCORES):
                    acc = 0
                    for t in ts:
                        tstart[(c, p, g, h, t)] = acc
                        acc += counts[(c, p, g, h, t)]
                    mx = max(mx, acc)
                nsec[p, g, h] = -(-mx // 128)
                assert nsec[p, g, h] >= 1

    # linear chunk ids: buckets (p, g) in p-major order; sections h0, h1
    bucket_base = {}
    sec_base = {}
    nch = 0
    for p in (0, 1):
        for g in range(NGRP):
            bucket_base[(p, g)] = nch
            sec_base[(p, g, 0)] = nch
            sec_base[(p, g, 1)] = nch + int(nsec[p, g, 0])
            nch += int(nsec[p, g, 0] + nsec[p, g, 1])

    # union chunk span (bucket-relative -> absolute) per (p, g, h, t)
    span = {}
    for p in (0, 1):
        for g, ts in enumerate(groups):
            for h in (0, 1):
                b = sec_base[(p, g, h)]
                for t in ts:
                    k0 = min(tstart[(c, p, g, h, t)] // 128
                             for c in range(NCORES))
                    k1 = max(-(-(tstart[(c, p, g, h, t)]
                                 + counts[(c, p, g, h, t)]) // 128)
                             for c in range(NCORES))
                    span[(p, g, h, t)] = (b + k0, b + max(k1, k0))

    # E-block ids in (p, g, h-section, chunk, tile) emission order
    eb_of = {}
    n_eb = 0
    bucket_mms = {}
    for p in (0, 1):
        for g, ts in enumerate(groups):
            mms = []
            for h in (0, 1):
                b = sec_base[(p, g, h)]
                for k in range(b, b + int(nsec[p, g, h])):
                    for t in ts:
                        k0, k1 = span[(p, g, h, t)]
                        if k0 <= k < k1:
                            eb_of[(k, t)] = n_eb
                            mms.append((k, t, n_eb))
                            n_eb += 1
            assert mms, f"empty bucket p={p} g={g}"
            bucket_mms[(p, g)] = mms
    ebarr = np.full((nch, GRP), -1, np.int64)
    for (k, t), eb in eb_of.items():
        ebarr[k, t % GRP] = eb

    def tile_close_chunk(g, t):
        """Chunk at which tile t closes (its last mm in bucket (1, g))."""
        cands = [k for (k, tt, _) in bucket_mms[(1, g)] if tt == t]
        return cands[-1] if cands else bucket_mms[(1, g)][-1][0]

    # ---- layer 1 ops: bucket pair order (0,g),(1,g); accumulate across ----
    ops1 = [[] for _ in range(nch)]
    for g in range(NGRP):
        mms0, mms1 = bucket_mms[(0, g)], bucket_mms[(1, g)]
        ops1[bucket_base[(0, g)]].append(("bank", g))
        for i, (k, t, eb) in enumerate(mms0):
            ops1[k].append(("mm", t, eb, i == 0, False))
        for i, (k, t, eb) in enumerate(mms1):
            ops1[k].append(("mm", t, eb, False, i == len(mms1) - 1))
        for t in groups[g]:
            ops1[tile_close_chunk(g, t)].append(("close", t, g))

    # ---- layer 2 ops: phase order; spill/reload SBUF partials ----
    ops2 = [[] for _ in range(nch)]
    for g in range(NGRP):
        mms0, mms1 = bucket_mms[(0, g)], bucket_mms[(1, g)]
        ops2[bucket_base[(0, g)]].append(("bank", g))
        for i, (k, t, eb) in enumerate(mms0):
            ops2[k].append(("mm", t, eb, i == 0, i == len(mms0) - 1))
        ops2[mms0[-1][0]].append(("pclose", g))
        ops2[bucket_base[(1, g)]].append(("reopen", g))
        for i, (k, t, eb) in enumerate(mms1):
            ops2[k].append(("mm", t, eb, False, i == len(mms1) - 1))
        for t in groups[g]:
            ops2[tile_close_chunk(g, t)].append(("close", t, g))

    kind_rank = {"bank": 0, "reopen": 0, "mm": 1, "pclose": 2, "close": 2}
    for ops in (ops1, ops2):
        for k in range(nch):
            ops[k].sort(key=lambda op: (kind_rank[op[0]],
                                        op[1] if op[0] == "close" else -1))

    # ---- gather blocks ----
    # layer 1: bucket PAIR order (0,g),(1,g) to match ops1's cross-bucket
    # bank accumulation; blocks per (p, g, h) section (source = Tp half h)
    blocks1 = []
    for g in range(NGRP):
        for p in (0, 1):
            for h in (0, 1):
                b = sec_base[(p, g, h)]
                nk = int(nsec[p, g, h])
                off = 0
                while off < nk:
                    n = min(GMAXC, nk - off)
                    blocks1.append([None, n, h, b + off])
                    off += n
    # layer 2: per (p, g) bucket (source = z1 piece p)
    blocks2 = []
    for p in (0, 1):
        for g in range(NGRP):
            b = bucket_base[(p, g)]
            nk = int(nsec[p, g, 0] + nsec[p, g, 1])
            off = 0
            while off < nk:
                n = min(GMAXC, nk - off)
                blocks2.append([None, n, p, b + off])
                off += n
    col = 0
    for blk in blocks1:
        blk[0] = col
        col += blk[1] * 8
    g1cols = col
    col = 0
    for blk in blocks2:
        blk[0] = col
        col += blk[1] * 8
    g2cols = col

    per_core = []
    for c in range(NCORES):
        # linear slot tables (chunk-id space)
        lin1 = np.zeros(nch * 128, np.int64)         # Tp half-space index
        lin2 = np.zeros(nch * 128, np.int64)         # z1 piece-space index
        dslot = np.zeros(nch * 128, np.float32)      # dinv[src] per slot
        dcole = np.full((128, n_eb), -1.0, np.float32)
        for p in (0, 1):
            for g, ts in enumerate(groups):
                for h in (0, 1):
                    b = sec_base[(p, g, h)]
                    for t in ts:
                        k = kid(c, p, g, h, t)
                        s0e, ne = int(starts[k]), counts[(c, p, g, h, t)]
                        if ne == 0:
                            continue
                        uidx = src_s[s0e : s0e + ne]
                        dcols = dcol_s[s0e : s0e + ne]
                        s0 = b * 128 + tstart[(c, p, g, h, t)]
                        lin1[s0 : s0 + ne] = tokp[uidx] - h * VLO
                        sl = uidx % NPAD
                        lin2[s0 : s0 + ne] = np.where(
                            sl < RA, (uidx // NPAD) * RA + sl,
                            (uidx // NPAD) * RB + (sl - RA))
                        dslot[s0 : s0 + ne] = dinvp[uidx]
                        gslots = (s0 - b * 128) + np.arange(ne)
                        ebs = ebarr[b + gslots // 128, t % GRP]
                        assert (ebs >= 0).all()
                        dcole[gslots % 128, ebs] = dcols
        gidx1 = np.concatenate(
            [_wrap_idx(lin1[b * 128 : (b + n) * 128])
             for (_, n, _, b) in blocks1], axis=1)
        gidx2 = np.concatenate(
            [_wrap_idx(lin2[b * 128 : (b + n) * 128])
             for (_, n, _, b) in blocks2], axis=1)
        dinvslot = np.ascontiguousarray(
            dslot.reshape(nch, 128).T)               # [128, nch]

        dv = dinvp[c * NPAD : (c + 1) * NPAD]
        dinv_loc = np.ascontiguousarray(dv.reshape(TPC, 128).T)

        emat = np.ascontiguousarray(
            (dcole[:, :, None] == np.arange(128, dtype=np.float32)[None, None, :])
            .astype(ml_dtypes.float8_e4m3).reshape(128, n_eb * 128))
        per_core.append({"gidx1": gidx1, "gidx2": gidx2, "emat": emat,
                         "dinvslot": dinvslot, "dinv": dinv_loc})

    layout = {"blocks1": blocks1, "blocks2": blocks2, "ops1": ops1,
              "ops2": ops2, "n_eb": n_eb, "nch": nch,
              "g1cols": g1cols, "g2cols": g2cols}
    return per_core, layout


def _build(layout):
    blocks1 = layout["blocks1"]
    blocks2 = layout["blocks2"]
    ops1 = layout["ops1"]
    ops2 = layout["ops2"]
    n_eb = layout["n_eb"]
    nch = layout["nch"]

    nc = bacc.Bacc("TRN2", target_bir_lowering=False, debug=False,
                   num_devices=NCORES, num_swdge_queues=NQ)

    tp_lo = nc.dram_tensor("tp_lo", [VLO, D], BF16, kind="ExternalInput")
    tp_hi = nc.dram_tensor("tp_hi", [V - VLO, D], BF16, kind="ExternalInput")
    g1_d = nc.dram_tensor("gidx1", [128, layout["g1cols"]], I16,
                          kind="ExternalInput")
    g2_d = nc.dram_tensor("gidx2", [128, layout["g2cols"]], I16,
                          kind="ExternalInput")
    emat_d = nc.dram_tensor("emat", [128, n_eb * 128], FP8,
                            kind="ExternalInput")
    dslot_d = nc.dram_tensor("dinvslot", [128, nch], F32,
                             kind="ExternalInput")
    dinv_d = nc.dram_tensor("dinv", [128, TPC], F32, kind="ExternalInput")
    w1t_d = nc.dram_tensor("w1t", [128, D], BF16, kind="ExternalInput")
    w2t_d = nc.dram_tensor("w2t", [128, D], BF16, kind="ExternalInput")
    bias_d = nc.dram_tensor("bias", [128, 2], F32, kind="ExternalInput")
    identb_d = nc.dram_tensor("identb", [128, 128], BF16, kind="ExternalInput")
    ident8_d = nc.dram_tensor("ident8", [128, 128], FP8, kind="ExternalInput")
    out_d = nc.dram_tensor("out", [NPAD, D], F32, kind="ExternalOutput")

    ACT = mybir.ActivationFunctionType

    with tile.TileContext(nc) as tc:
        with (
            tc.tile_pool(name="const", bufs=1) as cp,
            tc.tile_pool(name="msgs", bufs=MSGB) as msgp,
            tc.tile_pool(name="part", bufs=NGRP) as partp,
            tc.tile_pool(name="work", bufs=3) as wk,
            tc.tile_pool(name="stage", bufs=3) as stg,
            tc.tile_pool(name="psG", bufs=3, space="PSUM") as psG,
            tc.tile_pool(name="psT", bufs=2, space="PSUM") as psT,
            tc.tile_pool(name="psB", bufs=2, space="PSUM") as psB,
            tc.tile_pool(name="psC", bufs=1, space="PSUM") as psC,
            tc.tile_pool(name="dram", bufs=1, space="DRAM") as dram,
        ):
            nc.gpsimd.load_library(mlp)

            z1_a = dram.tile([RA, D], BF16)
            z1_b = dram.tile([RB, D], BF16)
            z1_fa = dram.tile([NCORES * RA, D], BF16, addr_space="Shared")
            z1_fb = dram.tile([NCORES * RB, D], BF16, addr_space="Shared")
            dum_l = dram.tile([16, D], BF16)
            dum_f = dram.tile([NCORES * 16, D], BF16, addr_space="Shared")

            def collective(z_loc, z_full):
                return nc.gpsimd.collective_compute(
                    "AllGather", mybir.AluOpType.bypass,
                    replica_groups=[list(range(NCORES))],
                    ins=[z_loc.opt()], outs=[z_full.opt()])

            # Warm up the collective fabric (first-collective barrier takes
            # 150-250us; run it concurrently with layer 1 from t=0).
            collective(dum_l, dum_f)

            g1_sb = cp.tile([128, layout["g1cols"]], I16)
            g2_sb = cp.tile([128, layout["g2cols"]], I16)
            emat_sb = cp.tile([128, n_eb, 128], FP8)
            dslot_sb = cp.tile([128, nch], F32)
            dinv_sb = cp.tile([128, TPC], F32)
            w1t_sb = cp.tile([128, D], BF16)
            w2t_sb = cp.tile([128, D], BF16)
            bias_sb = cp.tile([128, 2], F32)
            identb_sb = cp.tile([128, 128], BF16)
            ident8_sb = cp.tile([128, 128], FP8)
            gq = [0, layout["g1cols"] // 4, layout["g1cols"] // 2,
                  3 * layout["g1cols"] // 4, layout["g1cols"]]
            for qi in range(4):
                nc.sync.dma_start(g1_sb[:, gq[qi] : gq[qi + 1]],
                                  g1_d[:, gq[qi] : gq[qi + 1]])
            nc.sync.dma_start(dslot_sb[:], dslot_d[:])
            # E blocks in 4 slices so early matmuls start before the full
            # 12+MB load lands
            qs = [0, n_eb // 4, n_eb // 2, 3 * n_eb // 4, n_eb]
            for qi in range(4):
                a, b = qs[qi], qs[qi + 1]
                nc.sync.dma_start(
                    emat_sb[:, a:b, :],
                    emat_d[:, a * 128 : b * 128].rearrange(
                        "p (c f) -> p c f", f=128))
            nc.sync.dma_start(dinv_sb[:], dinv_d[:])
            nc.sync.dma_start(w1t_sb[:], w1t_d[:])
            nc.sync.dma_start(w2t_sb[:], w2t_d[:])
            nc.sync.dma_start(bias_sb[:], bias_d[:])
            nc.sync.dma_start(identb_sb[:], identb_d[:])
            nc.sync.dma_start(ident8_sb[:], ident8_d[:])
            nc.sync.dma_start(g2_sb[:], g2_d[:])

            qn = [0]

            def next_q():
                qn[0] = (qn[0] + 1) % NQ
                return qn[0]

            # Pre-touch the msgs buffers so pad slots never multiply
            # uninitialized SBUF into the PSUM accumulation.
            for _ in range(MSGB):
                mz = msgp.tile([128, GMAXC, D], BF16, name="m", tag="m")
                nc.vector.memset(mz[:], 0)

            def run_layer(blocks, ops, srcs, use_scale, wt_sb, bias_col,
                          relu, dest_a, dest_b, out_colls):
                open_ps = {}
                grp_ps = {}
                grp_part = {}
                grp_stage = {}
                grp_closed = {}

                def ntile_of(g):
                    return min(GRP, TPC - g * GRP)

                def op_bank(g, moving):
                    ntile = ntile_of(g)
                    ps = psG.tile([128, GRP * 128], F32, name="agg", tag="pG")
                    grp_ps[g] = ps
                    if moving is not None:
                        nc.tensor.matmul(
                            ps[:, 0 : ntile * 128], ident8_sb[:], moving,
                            start=True, stop=False, skip_group_check=True)
                    for j in range(ntile):
                        open_ps[g * GRP + j] = ps[:, j * 128 : (j + 1) * 128]
                    grp_stage[g] = stg.tile(
                        [128, ntile, D], F32 if dest_b is None else BF16,
                        name="stage1", tag="st1")
                    grp_closed[g] = 0

                def op_pclose(g):
                    ntile = ntile_of(g)
                    part = partp.tile([128, GRP, D], BF16, name="part",
                                      tag="part")
                    grp_part[g] = part
                    ps = grp_ps.pop(g)
                    nc.scalar.activation(
                        part[:, 0:ntile, :].rearrange("p t f -> p (t f)"),
                        ps[:, 0 : ntile * 128], ACT.Copy)
                    for j in range(ntile):
                        del open_ps[g * GRP + j]
                    del grp_stage[g], grp_closed[g]

                def op_reopen(g):
                    ntile = ntile_of(g)
                    part = grp_part.pop(g)
                    op_bank(g, part[:, 0:ntile, :]
                            .rearrange("p t f -> p (t f)"))

                def op_close(t, g):
                    ntile = ntile_of(g)
                    agg_sb = wk.tile([128, 128], BF16, name="agg_sb",
                                     tag="agg_sb")
                    nc.scalar.activation(agg_sb[:], open_ps.pop(t), ACT.Copy,
                                         scale=dinv_sb[:, t : t + 1])
                    aggT_ps = psT.tile([128, 128], BF16, name="aggT",
                                       tag="pT")
                    nc.tensor.matmul(aggT_ps[:], agg_sb[:], identb_sb[:],
                                     is_transpose=True, start=True, stop=True)
                    aggT_sb = wk.tile([128, 128], BF16, name="aggT_sb",
                                      tag="aggT_sb")
                    nc.scalar.activation(aggT_sb[:], aggT_ps[:], ACT.Copy)
                    yT_ps = psB.tile([128, 128], F32, name="yT", tag="pB")
                    nc.tensor.matmul(yT_ps[:], wt_sb[:], aggT_sb[:],
                                     start=True, stop=True)
                    yT_sb = wk.tile([128, 128], BF16, name="yT_sb",
                                    tag="yT_sb")
                    nc.scalar.activation(yT_sb[:], yT_ps[:],
                                         ACT.Relu if relu else ACT.Identity,
                                         bias=bias_col)
                    y_ps = psC.tile([128, 128], BF16, name="y", tag="pC")
                    nc.tensor.matmul(y_ps[:], yT_sb[:], identb_sb[:],
                                     is_transpose=True, start=True, stop=True)
                    gt0 = g * GRP
                    if dest_b is None:
                        nc.scalar.activation(grp_stage[g][:, t - gt0, :],
                                             y_ps[:], ACT.Copy)
                    else:
                        nc.scalar.activation(grp_stage[g][:, t - gt0, :],
                                             y_ps[:], ACT.Copy,
                                             scale=dinv_sb[:, t : t + 1])
                    grp_closed[g] += 1
                    if grp_closed[g] == ntile:
                        if dest_b is None:
                            dst_rows = dest_a[gt0 * 128
                                              : (gt0 + ntile) * 128, :]
                        elif gt0 < PA_T:
                            dst_rows = dest_a[gt0 * 128
                                              : (gt0 + ntile) * 128, :]
                        else:
                            dst_rows = dest_b[(gt0 - PA_T) * 128
                                              : (gt0 - PA_T + ntile) * 128, :]
                        nc.sync.dma_start(
                            dst_rows.rearrange("(t p) f -> p t f", p=128),
                            grp_stage[g][:])
                        del grp_ps[g], grp_stage[g], grp_closed[g]
                        if out_colls is not None:
                            if g == PA_T // GRP - 1:
                                out_colls[0]()
                            elif g == NGRP - 1:
                                out_colls[1]()

                gsb = g1_sb if use_scale else g2_sb
                for coloff, n, sid, base in blocks:
                    msgs = msgp.tile([128, GMAXC, D], BF16, name="m",
                                     tag="m")
                    nc.gpsimd.dma_gather(
                        msgs[:, 0:n, :], srcs[sid],
                        gsb[:, coloff : coloff + n * 8],
                        n * 128, n * 128, D, queue_num=next_q())
                    if use_scale:
                        # one broadcast multiply per gather block: slot ==
                        # partition, so dinv[src] is a per-partition scalar
                        # replicated along feats via a stride-0 AP
                        nc.vector.tensor_tensor(
                            msgs[:, 0:n, :], msgs[:, 0:n, :],
                            dslot_sb[:, base : base + n]
                            .to_broadcast((128, n, 128)),
                            mybir.AluOpType.mult)
                    for k in range(n):
                        ck = base + k
                        for op in ops[ck]:
                            if op[0] == "bank":
                                op_bank(op[1], None)
                            elif op[0] == "reopen":
                                op_reopen(op[1])
                            elif op[0] == "mm":
                                _, t, eb, st, sp = op
                                nc.tensor.matmul(
                                    open_ps[t], emat_sb[:, eb, :],
                                    msgs[:, k, :], start=st, stop=sp,
                                    skip_group_check=True)
                            elif op[0] == "pclose":
                                op_pclose(op[1])
                            elif op[0] == "close":
                                op_close(op[1], op[2])

            run_layer(blocks1, ops1, (tp_lo[:], tp_hi[:]), True, w1t_sb,
                      bias_sb[:, 0:1], True, z1_a, z1_b,
                      [lambda: collective(z1_a, z1_fa),
                       lambda: collective(z1_b, z1_fb)])
            run_layer(blocks2, ops2, (z1_fa[:], z1_fb[:]), False, w2t_sb,
                      bias_sb[:, 1:2], False, out_d.ap(), None, None)

    nc.compile()
    return nc


_CACHE = {}


def _run(inputs, trace=False):
    import ml_dtypes

    node_tokens = np.asarray(inputs["node_tokens"])
    edge_index = np.asarray(inputs["edge_index"])
    embed_table = np.asarray(inputs["embed_table"], dtype=np.float32)
    Wn = np.asarray(inputs["W_node_w"], dtype=np.float32)
    bn = np.asarray(inputs["W_node_b"], dtype=np.float32)
    w1 = np.asarray(inputs["w1"], dtype=np.float32)
    b1 = np.asarray(inputs["b1"], dtype=np.float32)
    w2 = np.asarray(inputs["w2"], dtype=np.float32)
    b2 = np.asarray(inputs["b2"], dtype=np.float32)

    per_core, layout = _preprocess(node_tokens, edge_index)

    if "nc" not in _CACHE:
        _CACHE["nc"] = _build(layout)
    nc = _CACHE["nc"]

    # Parameter-only preprocessing: fold the embedding projection.
    Tp = (embed_table @ Wn.T + bn).astype(ml_dtypes.bfloat16)   # [V, 128]
    tp_lo = Tp[:VLO]
    tp_hi = Tp[VLO:]
    bias = np.stack([b1, b2], axis=1).astype(np.float32)
    identb = np.eye(128, dtype=ml_dtypes.bfloat16)
    ident8 = np.eye(128, dtype=ml_dtypes.float8_e4m3)


    in_maps = []
    for c in range(NCORES):
        in_maps.append({
            "tp_lo": tp_lo, "tp_hi": tp_hi,
            "gidx1": per_core[c]["gidx1"],
            "gidx2": per_core[c]["gidx2"],
            "emat": per_core[c]["emat"],
            "dinvslot": per_core[c]["dinvslot"],
            "dinv": per_core[c]["dinv"],
            "w1t": w1.T.astype(ml_dtypes.bfloat16),
            "w2t": w2.T.astype(ml_dtypes.bfloat16),
            "bias": bias, "identb": identb, "ident8": ident8,
        })

    res = run_bass_kernel_spmd(nc, in_maps, core_ids=list(range(NCORES)),
                               trace=trace)
    out = np.concatenate([res.results[c]["out"][:NPC] for c in range(NCORES)],
                         axis=0)
    return out.astype(np.float32), res


def kernel(**inputs):
    out, _ = _run(inputs, trace=False)
    return out


# revision 15
# speedup vs baseline: 1.1528x; 1.0142x over previous
"""Distributed 2-layer GCN on 8 TRN2 NeuronCores (Bass/Tile).

Reference computation (PyG-style GCNConv, f32):
    e  = embed_table[node_tokens]            # [N, 256]
    x0 = e @ Wn^T + bn                       # [N, 128]
    h1 = Ahat @ (x0 @ w1^T) + b1 ; z1 = relu(h1)
    h2 = Ahat @ (z1 @ w2^T) + b2             # output [N, 128]
  with Ahat = D^-1/2 (A + I) D^-1/2, deg from dst(+self loops).

Sharding: nodes are partitioned contiguously across the 8 cores (6250 each,
padded to 6272 = 49 tiles of 128). Each core aggregates the edges pointing
at its own nodes, projects, and writes its output shard.

v7 design:
  - The embedding lookup + input projection is folded on the host into a
    projected table Tp = embed_table @ Wn^T + bn  [V, 128] bf16 (parameter-
    only preprocessing, stored as lo/hi halves for int16 gather indexing).
  - LAYER 1 NEEDS NO COLLECTIVE AT ALL: its messages are gathered straight
    from the replicated Tp (idx = tok[src]); the per-src norm dinv[src] is
    applied per chunk on the (otherwise idle) vector engine via a
    per-partition tensor_scalar multiply (slot == partition). Self loops
    are ordinary slots (idx = tok[i], scale dinv[i]). Layer 1 therefore
    starts at t~0, fully overlapping the fabric's first-collective barrier
    (~150-250us), which a dummy AllGather kicks off immediately.
  - z1 is sharded into TWO pieces per core: piece A = tiles 0..23, piece B
    = tiles 24..48; each piece is AllGathered as soon as its rows close
    (AG_a overlaps layer 1's tail, AG_b overlaps layer 2's piece-A phase).
    The 8*3072 / 8*3200-row gathered pieces each fit int16 index space.
  - ONE shared slot/E-matrix layout serves both layers: edge slots are
    bucketed per (src-node piece, dst-tile group of 4), with each bucket
    split into (src-token half) sections chunk-aligned so every 128-slot
    chunk has a single gather source in BOTH address spaces (Tp halves for
    layer 1, z1 pieces for layer 2). Slots are sorted by (half, dst tile,
    src) and deduped per (dst tile, src); E blocks are 0/1 multiplicity
    counts, exact in fp8, resident in SBUF, reused by both layers. Layer 2
    gathers the same slots from z1 (self slots read the core's own rows,
    already dinv-scaled, so no vector multiply).
  - Layer 1 processes buckets in (group: piece A then B) pair order,
    accumulating each 4-tile PSUM bank across both buckets. Layer 2
    processes all piece-A buckets first (so they only wait on AG_a),
    spilling each bank to an SBUF partial, then reopens from the partial
    during the piece-B phase.
  - Per dst tile close: agg[dst,feat] -> copy*dinv[dst] -> transpose ->
    w^T matmul -> bias(+relu) -> transpose -> store copy (*dinv for z1's
    pre-scale; plain f32 for the final output) -> one DMA per group ->
    piece AllGather dispatch after groups 5 / 12.
"""

import os

import numpy as np

import concourse.bacc as bacc
from bass_rust import InstructionNameOrderedSet
import concourse.mybir as mybir
import concourse.tile as tile
from concourse.bass_utils import run_bass_kernel_spmd
from concourse.library_config import mlp

# Problem shape (hardcoded per harness contract)
N = 50000
E = 600000
V = 50000
D_IN = 256
D = 128
NCORES = 8

NPC = N // NCORES            # 6250 nodes per core
TPC = (NPC + 127) // 128     # 49 tiles per core
NPAD = TPC * 128             # 6272 padded nodes per core
PA_T = 24                    # piece A: tiles 0..23
PB_T = TPC - PA_T            # piece B: tiles 24..48
RA = PA_T * 128              # 3072 rows per core in piece A
RB = PB_T * 128              # 3200 rows per core in piece B
VLO = V // 2                 # 25000: projected-table split
GRP = 4                      # dst tiles per aggregation group (PSUM bank)
NGRP = (TPC + GRP - 1) // GRP
GMAXC = 8                    # max chunks (x128 slots) per dma_gather
NQ = 4                       # SWDGE queues
F32 = mybir.dt.float32
BF16 = mybir.dt.bfloat16
FP8 = mybir.dt.float8e4
I16 = mybir.dt.int16
MSGB = int(os.environ.get("KMSGB", "12"))   # msgs pool bufs


def _wrap_idx(idx_linear):
    """[n] -> [128, n/16] int16: position j at [j%16, j//16], replicated x8."""
    n = idx_linear.shape[0]
    assert n % 16 == 0
    w = idx_linear.astype(np.int16).reshape(-1, 16).T
    return np.tile(w, (8, 1))


def _preprocess(node_tokens, edge_index):
    """Build per-core host arrays + the (core-uniform) schedules.

    Shared slot layout: buckets (piece p, group g), each split into two
    chunk-aligned sections by src-token half h. Within a section, slots
    sorted by (dst tile, src), deduped per (dst tile, src). Self edges
    (i -> i) are included for every padded node.

    Returns per-core {gidx1, gidx2, emat, dinvslot, dinv} plus layout:
      blocks1/blocks2: [(gidx_col_off, n_chunks, src_id, chunk_base)]
        src_id: layer 1 -> 0=tp_lo 1=tp_hi; layer 2 -> 0=z_fa 1=z_fb
      ops1/ops2: per linear chunk, ordered ops:
        ("bank", g) | ("reopen", g) | ("mm", t, eb, start, stop) |
        ("pclose", g) | ("close", t, g)
      n_eb, nch: E-block and chunk counts
    """
    import ml_dtypes

    src_e = np.asarray(edge_index[0], dtype=np.int64)
    dst_e = np.asarray(edge_index[1], dtype=np.int64)
    tok = np.asarray(node_tokens, dtype=np.int64)

    deg = (np.bincount(dst_e, minlength=N) + 1).astype(np.float32)
    dinv = (1.0 / np.sqrt(deg)).astype(np.float32)
    # per padded-node-id token / dinv (pads: tok 0, dinv 0)
    tokp = np.zeros(NCORES * NPAD, np.int64)
    dinvp = np.zeros(NCORES * NPAD, np.float32)
    for c in range(NCORES):
        tokp[c * NPAD : c * NPAD + NPC] = tok[c * NPC : (c + 1) * NPC]
        dinvp[c * NPAD : c * NPAD + NPC] = dinv[c * NPC : (c + 1) * NPC]

    # edge list in padded-node-id space + self loops for every padded node
    pid = lambda n: (n // NPC) * NPAD + (n % NPC)
    allid = np.arange(NCORES * NPAD)
    src = np.concatenate([pid(src_e), allid])
    dst = np.concatenate([pid(dst_e), allid])

    core = dst // NPAD
    dloc = dst % NPAD
    tloc = dloc // 128
    dcol = (dloc % 128).astype(np.int64)
    sloc = src % NPAD
    piece = (sloc >= RA).astype(np.int64)
    half = (tokp[src] >= VLO).astype(np.int64)
    grp = tloc // GRP

    # sort per (dst core, piece, group, half, dst tile, src)
    key = ((((core * 2 + piece) * NGRP + grp) * 2 + half) * TPC + tloc)
    order = np.lexsort((src, key))
    key_s = key[order]
    src_s = src[order]
    dcol_s = dcol[order]
    nkey = NCORES * 2 * NGRP * 2 * TPC
    counts_raw = np.bincount(key_s, minlength=nkey)
    starts = np.zeros(nkey + 1, dtype=np.int64)
    np.cumsum(counts_raw, out=starts[1:])

    def kid(c, p, g, h, t):
        return (((c * 2 + p) * NGRP + g) * 2 + h) * TPC + t

    groups = [list(range(g * GRP, min((g + 1) * GRP, TPC)))
              for g in range(NGRP)]

    dedup = {}
    counts = {}
    for c in range(NCORES):
        for p in (0, 1):
            for g, ts in enumerate(groups):
                for h in (0, 1):
                    for t in ts:
                        k = kid(c, p, g, h, t)
                        s0, ne = starts[k], int(counts_raw[k])
                        uidx, inv = np.unique(src_s[s0 : s0 + ne],
                                              return_inverse=True)
                        dedup[(c, p, g, h, t)] = (uidx, inv,
                                                  dcol_s[s0 : s0 + ne])
                        counts[(c, p, g, h, t)] = uidx.shape[0]

    # section chunk counts (maxed over cores) and per-core slot offsets
    nsec = np.zeros((2, NGRP, 2), np.int64)
    tstart = {}
    for p in (0, 1):
        for g, ts in enumerate(groups):
            for h in (0, 1):
                mx = 0
                for c in range(NCORES):
                    acc = 0
                    for t in ts:
                        tstart[(c, p, g, h, t)] = acc
                        acc += counts[(c, p, g, h, t)]
                    mx = max(mx, acc)
                nsec[p, g, h] = -(-mx // 128)
                assert nsec[p, g, h] >= 1

    # linear chunk ids: buckets (p, g) in p-major order; sections h0, h1
    bucket_base = {}
    sec_base = {}
    nch = 0
    for p in (0, 1):
        for g in range(NGRP):
            bucket_base[(p, g)] = nch
            sec_base[(p, g, 0)] = nch
            sec_base[(p, g, 1)] = nch + int(nsec[p, g, 0])
            nch += int(nsec[p, g, 0] + nsec[p, g, 1])

    # union chunk span (bucket-relative -> absolute) per (p, g, h, t)
    span = {}
    for p in (0, 1):
        for g, ts in enumerate(groups):
            for h in (0, 1):
                b = sec_base[(p, g, h)]
                for t in ts:
                    k0 = min(tstart[(c, p, g, h, t)] // 128
                             for c in range(NCORES))
                    k1 = max(-(-(tstart[(c, p, g, h, t)]
                                 + counts[(c, p, g, h, t)]) // 128)
                             for c in range(NCORES))
                    span[(p, g, h, t)] = (b + k0, b + max(k1, k0))

    # E-block ids in (p, g, h-section, chunk, tile) emission order
    eb_of = {}
    n_eb = 0
    bucket_mms = {}
    for p in (0, 1):
        for g, ts in enumerate(groups):
            mms = []
            for h in (0, 1):
                b = sec_base[(p, g, h)]
                for k in range(b, b + int(nsec[p, g, h])):
                    for t in ts:
                        k0, k1 = span[(p, g, h, t)]
                        if k0 <= k < k1:
                            eb_of[(k, t)] = n_eb
                            mms.append((k, t, n_eb))
                            n_eb += 1
            assert mms, f"empty bucket p={p} g={g}"
            bucket_mms[(p, g)] = mms
    ebarr = np.full((nch, GRP), -1, np.int64)
    for (k, t), eb in eb_of.items():
        ebarr[k, t % GRP] = eb

    def tile_close_chunk(g, t):
        """Chunk at which tile t closes (its last mm in bucket (1, g))."""
        cands = [k for (k, tt, _) in bucket_mms[(1, g)] if tt == t]
        return cands[-1] if cands else bucket_mms[(1, g)][-1][0]

    # ---- layer 1 ops: bucket pair order (0,g),(1,g); accumulate across ----
    ops1 = [[] for _ in range(nch)]
    for g in range(NGRP):
        mms0, mms1 = bucket_mms[(0, g)], bucket_mms[(1, g)]
        ops1[bucket_base[(0, g)]].append(("bank", g))
        for i, (k, t, eb) in enumerate(mms0):
            ops1[k].append(("mm", t, eb, i == 0, False))
        for i, (k, t, eb) in enumerate(mms1):
            ops1[k].append(("mm", t, eb, False, i == len(mms1) - 1))
        for t in groups[g]:
            ops1[tile_close_chunk(g, t)].append(("close", t, g))

    # ---- layer 2 ops: phase order; spill/reload SBUF partials ----
    ops2 = [[] for _ in range(nch)]
    for g in range(NGRP):
        mms0, mms1 = bucket_mms[(0, g)], bucket_mms[(1, g)]
        ops2[bucket_base[(0, g)]].append(("bank", g))
        for i, (k, t, eb) in enumerate(mms0):
            ops2[k].append(("mm", t, eb, i == 0, i == len(mms0) - 1))
        ops2[mms0[-1][0]].append(("pclose", g))
        ops2[bucket_base[(1, g)]].append(("reopen", g))
        for i, (k, t, eb) in enumerate(mms1):
            ops2[k].append(("mm", t, eb, False, i == len(mms1) - 1))
        for t in groups[g]:
            ops2[tile_close_chunk(g, t)].append(("close", t, g))

    kind_rank = {"bank": 0, "reopen": 0, "mm": 1, "pclose": 2, "close": 2}
    for ops in (ops1, ops2):
        for k in range(nch):
            ops[k].sort(key=lambda op: (kind_rank[op[0]],
                                        op[1] if op[0] == "close" else -1))

    # ---- gather blocks ----
    # layer 1: bucket PAIR order (0,g),(1,g) to match ops1's cross-bucket
    # bank accumulation; blocks per (p, g, h) section (source = Tp half h)
    blocks1 = []
    for g in range(NGRP):
        for p in (0, 1):
            for h in (0, 1):
                b = sec_base[(p, g, h)]
                nk = int(nsec[p, g, h])
                off = 0
                while off < nk:
                    n = min(GMAXC, nk - off)
                    blocks1.append([None, n, h, b + off])
                    off += n
    # layer 2: per (p, g) bucket (source = z1 piece p)
    blocks2 = []
    for p in (0, 1):
        for g in range(NGRP):
            b = bucket_base[(p, g)]
            nk = int(nsec[p, g, 0] + nsec[p, g, 1])
            off = 0
            while off < nk:
                n = min(GMAXC, nk - off)
                blocks2.append([None, n, p, b + off])
                off += n
    col = 0
    for blk in blocks1:
        blk[0] = col
        col += blk[1] * 8
    g1cols = col
    col = 0
    for blk in blocks2:
        blk[0] = col
        col += blk[1] * 8
    g2cols = col

    per_core = []
    for c in range(NCORES):
        # linear slot tables (chunk-id space)
        lin1 = np.zeros(nch * 128, np.int64)         # Tp half-space index
        lin2 = np.zeros(nch * 128, np.int64)         # z1 piece-space index
        dslot = np.zeros(nch * 128, np.float32)      # dinv[src] per slot
        eflat = np.zeros((n_eb, 128, 128), np.float32)
        for p in (0, 1):
            for g, ts in enumerate(groups):
                for h in (0, 1):
                    b = sec_base[(p, g, h)]
                    for t in ts:
                        uidx, inv, dcols = dedup[(c, p, g, h, t)]
                        ne = uidx.shape[0]
                        if ne == 0:
                            continue
                        s0 = b * 128 + tstart[(c, p, g, h, t)]
                        lin1[s0 : s0 + ne] = tokp[uidx] - h * VLO
                        sl = uidx % NPAD
                        lin2[s0 : s0 + ne] = np.where(
                            sl < RA, (uidx // NPAD) * RA + sl,
                            (uidx // NPAD) * RB + (sl - RA))
                        dslot[s0 : s0 + ne] = dinvp[uidx]
                        gslots = (s0 - b * 128) + inv
                        ebs = ebarr[b + gslots // 128, t % GRP]
                        assert (ebs >= 0).all()
                        np.add.at(eflat, (ebs, gslots % 128, dcols), 1.0)
        gidx1 = np.concatenate(
            [_wrap_idx(lin1[b * 128 : (b + n) * 128])
             for (_, n, _, b) in blocks1], axis=1)
        gidx2 = np.concatenate(
            [_wrap_idx(lin2[b * 128 : (b + n) * 128])
             for (_, n, _, b) in blocks2], axis=1)
        dinvslot = np.ascontiguousarray(
            dslot.reshape(nch, 128).T)               # [128, nch]

        dv = dinvp[c * NPAD : (c + 1) * NPAD]
        dinv_loc = np.ascontiguousarray(dv.reshape(TPC, 128).T)

        emat = np.ascontiguousarray(
            eflat.transpose(1, 0, 2).reshape(128, n_eb * 128)
        ).astype(ml_dtypes.float8_e4m3)
        per_core.append({"gidx1": gidx1, "gidx2": gidx2, "emat": emat,
                         "dinvslot": dinvslot, "dinv": dinv_loc})

    layout = {"blocks1": blocks1, "blocks2": blocks2, "ops1": ops1,
              "ops2": ops2, "n_eb": n_eb, "nch": nch,
              "g1cols": g1cols, "g2cols": g2cols}
    return per_core, layout


def _build(layout):
    blocks1 = layout["blocks1"]
    blocks2 = layout["blocks2"]
    ops1 = layout["ops1"]
    ops2 = layout["ops2"]
    n_eb = layout["n_eb"]
    nch = layout["nch"]

    nc = bacc.Bacc("TRN2", target_bir_lowering=False, debug=False,
                   num_devices=NCORES, num_swdge_queues=NQ)

    tp_lo = nc.dram_tensor("tp_lo", [VLO, D], BF16, kind="ExternalInput")
    tp_hi = nc.dram_tensor("tp_hi", [V - VLO, D], BF16, kind="ExternalInput")
    g1_d = nc.dram_tensor("gidx1", [128, layout["g1cols"]], I16,
                          kind="ExternalInput")
    g2_d = nc.dram_tensor("gidx2", [128, layout["g2cols"]], I16,
                          kind="ExternalInput")
    emat_d = nc.dram_tensor("emat", [128, n_eb * 128], FP8,
                            kind="ExternalInput")
    dslot_d = nc.dram_tensor("dinvslot", [128, nch], F32,
                             kind="ExternalInput")
    dinv_d = nc.dram_tensor("dinv", [128, TPC], F32, kind="ExternalInput")
    w1t_d = nc.dram_tensor("w1t", [128, D], BF16, kind="ExternalInput")
    w2t_d = nc.dram_tensor("w2t", [128, D], BF16, kind="ExternalInput")
    bias_d = nc.dram_tensor("bias", [128, 2], F32, kind="ExternalInput")
    identb_d = nc.dram_tensor("identb", [128, 128], BF16, kind="ExternalInput")
    ident8_d = nc.dram_tensor("ident8", [128, 128], FP8, kind="ExternalInput")
    out_d = nc.dram_tensor("out", [NPAD, D], F32, kind="ExternalOutput")

    ACT = mybir.ActivationFunctionType

    with tile.TileContext(nc) as tc:
        with (
            tc.tile_pool(name="const", bufs=1) as cp,
            tc.tile_pool(name="msgs", bufs=MSGB) as msgp,
            tc.tile_pool(name="part", bufs=NGRP) as partp,
            tc.tile_pool(name="work", bufs=3) as wk,
            tc.tile_pool(name="stage", bufs=3) as stg,
            tc.tile_pool(name="psG", bufs=3, space="PSUM") as psG,
            tc.tile_pool(name="psT", bufs=2, space="PSUM") as psT,
            tc.tile_pool(name="psB", bufs=2, space="PSUM") as psB,
            tc.tile_pool(name="psC", bufs=1, space="PSUM") as psC,
            tc.tile_pool(name="dram", bufs=1, space="DRAM") as dram,
        ):
            nc.gpsimd.load_library(mlp)

            z1_a = dram.tile([RA, D], BF16)
            z1_b = dram.tile([RB, D], BF16)
            z1_fa = dram.tile([NCORES * RA, D], BF16, addr_space="Shared")
            z1_fb = dram.tile([NCORES * RB, D], BF16, addr_space="Shared")
            dum_l = dram.tile([16, D], BF16)
            dum_f = dram.tile([NCORES * 16, D], BF16, addr_space="Shared")

            def collective(z_loc, z_full):
                return nc.gpsimd.collective_compute(
                    "AllGather", mybir.AluOpType.bypass,
                    replica_groups=[list(range(NCORES))],
                    ins=[z_loc.opt()], outs=[z_full.opt()])

            # Warm up the collective fabric (first-collective barrier takes
            # 150-250us; run it concurrently with layer 1 from t=0).
            collective(dum_l, dum_f)

            g1_sb = cp.tile([128, layout["g1cols"]], I16)
            g2_sb = cp.tile([128, layout["g2cols"]], I16)
            emat_sb = cp.tile([128, n_eb, 128], FP8)
            dslot_sb = cp.tile([128, nch], F32)
            dinv_sb = cp.tile([128, TPC], F32)
            w1t_sb = cp.tile([128, D], BF16)
            w2t_sb = cp.tile([128, D], BF16)
            bias_sb = cp.tile([128, 2], F32)
            identb_sb = cp.tile([128, 128], BF16)
            ident8_sb = cp.tile([128, 128], FP8)
            gq = [0, layout["g1cols"] // 4, layout["g1cols"] // 2,
                  3 * layout["g1cols"] // 4, layout["g1cols"]]
            for qi in range(4):
                nc.sync.dma_start(g1_sb[:, gq[qi] : gq[qi + 1]],
                                  g1_d[:, gq[qi] : gq[qi + 1]])
            nc.sync.dma_start(dslot_sb[:], dslot_d[:])
            # E blocks in 4 slices so early matmuls start before the full
            # 12+MB load lands
            qs = [0, n_eb // 4, n_eb // 2, 3 * n_eb // 4, n_eb]
            for qi in range(4):
                a, b = qs[qi], qs[qi + 1]
                nc.sync.dma_start(
                    emat_sb[:, a:b, :],
                    emat_d[:, a * 128 : b * 128].rearrange(
                        "p (c f) -> p c f", f=128))
            nc.sync.dma_start(dinv_sb[:], dinv_d[:])
            nc.sync.dma_start(w1t_sb[:], w1t_d[:])
            nc.sync.dma_start(w2t_sb[:], w2t_d[:])
            nc.sync.dma_start(bias_sb[:], bias_d[:])
            nc.sync.dma_start(identb_sb[:], identb_d[:])
            nc.sync.dma_start(ident8_sb[:], ident8_d[:])
            nc.sync.dma_start(g2_sb[:], g2_d[:])

            qn = [0]

            def next_q():
                qn[0] = (qn[0] + 1) % NQ
                return qn[0]

            # Pre-touch the msgs buffers so pad slots never multiply
            # uninitialized SBUF into the PSUM accumulation.
            for _ in range(MSGB):
                mz = msgp.tile([128, GMAXC, D], BF16, name="m", tag="m")
                nc.vector.memset(mz[:], 0)

            def run_layer(blocks, ops, srcs, use_scale, wt_sb, bias_col,
                          relu, dest_a, dest_b, out_colls):
                open_ps = {}
                grp_ps = {}
                grp_part = {}
                grp_stage = {}
                grp_closed = {}

                def ntile_of(g):
                    return min(GRP, TPC - g * GRP)

                def op_bank(g, moving):
                    ntile = ntile_of(g)
                    ps = psG.tile([128, GRP * 128], F32, name="agg", tag="pG")
                    grp_ps[g] = ps
                    if moving is not None:
                        nc.tensor.matmul(
                            ps[:, 0 : ntile * 128], ident8_sb[:], moving,
                            start=True, stop=False, skip_group_check=True)
                    for j in range(ntile):
                        open_ps[g * GRP + j] = ps[:, j * 128 : (j + 1) * 128]
                    grp_stage[g] = stg.tile(
                        [128, ntile, D], F32 if dest_b is None else BF16,
                        name="stage1", tag="st1")
                    grp_closed[g] = 0

                def op_pclose(g):
                    ntile = ntile_of(g)
                    part = partp.tile([128, GRP, D], BF16, name="part",
                                      tag="part")
                    grp_part[g] = part
                    ps = grp_ps.pop(g)
                    nc.scalar.activation(
                        part[:, 0:ntile, :].rearrange("p t f -> p (t f)"),
                        ps[:, 0 : ntile * 128], ACT.Copy)
                    for j in range(ntile):
                        del open_ps[g * GRP + j]
                    del grp_stage[g], grp_closed[g]

                def op_reopen(g):
                    ntile = ntile_of(g)
                    part = grp_part.pop(g)
                    op_bank(g, part[:, 0:ntile, :]
                            .rearrange("p t f -> p (t f)"))

                def op_close(t, g):
                    ntile = ntile_of(g)
                    agg_sb = wk.tile([128, 128], BF16, name="agg_sb",
                                     tag="agg_sb")
                    nc.scalar.activation(agg_sb[:], open_ps.pop(t), ACT.Copy,
                                         scale=dinv_sb[:, t : t + 1])
                    aggT_ps = psT.tile([128, 128], BF16, name="aggT",
                                       tag="pT")
                    nc.tensor.matmul(aggT_ps[:], agg_sb[:], identb_sb[:],
                                     is_transpose=True, start=True, stop=True)
                    aggT_sb = wk.tile([128, 128], BF16, name="aggT_sb",
                                      tag="aggT_sb")
                    nc.scalar.activation(aggT_sb[:], aggT_ps[:], ACT.Copy)
                    yT_ps = psB.tile([128, 128], F32, name="yT", tag="pB")
                    nc.tensor.matmul(yT_ps[:], wt_sb[:], aggT_sb[:],
                                     start=True, stop=True)
                    yT_sb = wk.tile([128, 128], BF16, name="yT_sb",
                                    tag="yT_sb")
                    nc.scalar.activation(yT_sb[:], yT_ps[:],
                                         ACT.Relu if relu else ACT.Identity,
                                         bias=bias_col)
                    y_ps = psC.tile([128, 128], BF16, name="y", tag="pC")
                    nc.tensor.matmul(y_ps[:], yT_sb[:], identb_sb[:],
                                     is_transpose=True, start=True, stop=True)
                    gt0 = g * GRP
                    if dest_b is None:
                        nc.scalar.activation(grp_stage[g][:, t - gt0, :],
                                             y_ps[:], ACT.Copy)
                    else:
                        nc.scalar.activation(grp_stage[g][:, t - gt0, :],
                                             y_ps[:], ACT.Copy,
                                             scale=dinv_sb[:, t : t + 1])
                    grp_closed[g] += 1
                    if grp_closed[g] == ntile:
                        if dest_b is None:
                            dst_rows = dest_a[gt0 * 128
                                              : (gt0 + ntile) * 128, :]
                        elif gt0 < PA_T:
                            dst_rows = dest_a[gt0 * 128
                                              : (gt0 + ntile) * 128, :]
                        else:
                            dst_rows = dest_b[(gt0 - PA_T) * 128
                                              : (gt0 - PA_T + ntile) * 128, :]
                        nc.sync.dma_start(
                            dst_rows.rearrange("(t p) f -> p t f", p=128),
                            grp_stage[g][:])
                        del grp_ps[g], grp_stage[g], grp_closed[g]
                        if out_colls is not None:
                            if g == PA_T // GRP - 1:
                                out_colls[0]()
                            elif g == NGRP - 1:
                                out_colls[1]()

                gsb = g1_sb if use_scale else g2_sb
                for coloff, n, sid, base in blocks:
                    msgs = msgp.tile([128, GMAXC, D], BF16, name="m",
                                     tag="m")
                    nc.gpsimd.dma_gather(
                        msgs[:, 0:n, :], srcs[sid],
                        gsb[:, coloff : coloff + n * 8],
                        n * 128, n * 128, D, queue_num=next_q())
                    if use_scale:
                        # one broadcast multiply per gather block: slot ==
                        # partition, so dinv[src] is a per-partition scalar
                        # replicated along feats via a stride-0 AP
                        nc.vector.tensor_tensor(
                            msgs[:, 0:n, :], msgs[:, 0:n, :],
                            dslot_sb[:, base : base + n]
                            .to_broadcast((128, n, 128)),
                            mybir.AluOpType.mult)
                    for k in range(n):
                        ck = base + k
                        for op in ops[ck]:
                            if op[0] == "bank":
                                op_bank(op[1], None)
                            elif op[0] == "reopen":
                                op_reopen(op[1])
                            elif op[0] == "mm":
                                _, t, eb, st, sp = op
                                nc.tensor.matmul(
                                    open_ps[t], emat_sb[:, eb, :],
                                    msgs[:, k, :], start=st, stop=sp,
                                    skip_group_check=True)
                            elif op[0] == "pclose":
                                op_pclose(op[1])
                            elif op[0] == "close":
                                op_close(op[1], op[2])

            run_layer(blocks1, ops1, (tp_lo[:], tp_hi[:]), True, w1t_sb,
                      bias_sb[:, 0:1], True, z1_a, z1_b,
                      [lambda: collective(z1_a, z1_fa),
                       lambda: collective(z1_b, z1_fb)])
            run_layer(blocks2, ops2, (z1_fa[:], z1_fb[:]), False, w2t_sb,
                      bias_sb[:, 1:2], False, out_d.ap(), None, None)

    nc.compile()
    return nc


_CACHE = {}


def _run(inputs, trace=False):
    import ml_dtypes

    node_tokens = np.asarray(inputs["node_tokens"])
    edge_index = np.asarray(inputs["edge_index"])
    embed_table = np.asarray(inputs["embed_table"], dtype=np.float32)
    Wn = np.asarray(inputs["W_node_w"], dtype=np.float32)
    bn = np.asarray(inputs["W_node_b"], dtype=np.float32)
    w1 = np.asarray(inputs["w1"], dtype=np.float32)
    b1 = np.asarray(inputs["b1"], dtype=np.float32)
    w2 = np.asarray(inputs["w2"], dtype=np.float32)
    b2 = np.asarray(inputs["b2"], dtype=np.float32)

    per_core, layout = _preprocess(node_tokens, edge_index)

    if "nc" not in _CACHE:
        _CACHE["nc"] = _build(layout)
    nc = _CACHE["nc"]

    # Parameter-only preprocessing: fold the embedding projection.
    Tp = (embed_table @ Wn.T + bn).astype(ml_dtypes.bfloat16)   # [V, 128]
    tp_lo = Tp[:VLO]
    tp_hi = Tp[VLO:]
    bias = np.stack([b1, b2], axis=1).astype(np.float32)
    identb = np.eye(128, dtype=ml_dtypes.bfloat16)
    ident8 = np.eye(128, dtype=ml_dtypes.float8_e4m3)


    in_maps = []
    for c in range(NCORES):
        in_maps.append({
            "tp_lo": tp_lo, "tp_hi": tp_hi,
            "gidx1": per_core[c]["gidx1"],
            "gidx2": per_core[c]["gidx2"],
            "emat": per_core[c]["emat"],
            "dinvslot": per_core[c]["dinvslot"],
            "dinv": per_core[c]["dinv"],
            "w1t": w1.T.astype(ml_dtypes.bfloat16),
            "w2t": w2.T.astype(ml_dtypes.bfloat16),
            "bias": bias, "identb": identb, "ident8": ident8,
        })

    res = run_bass_kernel_spmd(nc, in_maps, core_ids=list(range(NCORES)),
                               trace=trace)
    out = np.concatenate([res.results[c]["out"][:NPC] for c in range(NCORES)],
                         axis=0)
    return out.astype(np.float32), res


def kernel(**inputs):
    out, _ = _run(inputs, trace=False)
    return out


# revision 16
# speedup vs baseline: 1.2077x; 1.0476x over previous
"""Distributed 2-layer GCN on 8 TRN2 NeuronCores (Bass/Tile).

Reference computation (PyG-style GCNConv, f32):
    e  = embed_table[node_tokens]            # [N, 256]
    x0 = e @ Wn^T + bn                       # [N, 128]
    h1 = Ahat @ (x0 @ w1^T) + b1 ; z1 = relu(h1)
    h2 = Ahat @ (z1 @ w2^T) + b2             # output [N, 128]
  with Ahat = D^-1/2 (A + I) D^-1/2, deg from dst(+self loops).

Sharding: nodes are partitioned contiguously across the 8 cores (6250 each,
padded to 6272 = 49 tiles of 128). Each core aggregates the edges pointing
at its own nodes, projects, and writes its output shard.

v7 design:
  - The embedding lookup + input projection is folded on the host into a
    projected table Tp = embed_table @ Wn^T + bn  [V, 128] bf16 (parameter-
    only preprocessing, stored as lo/hi halves for int16 gather indexing).
  - LAYER 1 NEEDS NO COLLECTIVE AT ALL: its messages are gathered straight
    from the replicated Tp (idx = tok[src]); the per-src norm dinv[src] is
    applied per chunk on the (otherwise idle) vector engine via a
    per-partition tensor_scalar multiply (slot == partition). Self loops
    are ordinary slots (idx = tok[i], scale dinv[i]). Layer 1 therefore
    starts at t~0, fully overlapping the fabric's first-collective barrier
    (~150-250us), which a dummy AllGather kicks off immediately.
  - z1 is sharded into TWO pieces per core: piece A = tiles 0..23, piece B
    = tiles 24..48; each piece is AllGathered as soon as its rows close
    (AG_a overlaps layer 1's tail, AG_b overlaps layer 2's piece-A phase).
    The 8*3072 / 8*3200-row gathered pieces each fit int16 index space.
  - ONE shared slot/E-matrix layout serves both layers: edge slots are
    bucketed per (src-node piece, dst-tile group of 4), with each bucket
    split into (src-token half) sections chunk-aligned so every 128-slot
    chunk has a single gather source in BOTH address spaces (Tp halves for
    layer 1, z1 pieces for layer 2). Slots are sorted by (half, dst tile,
    src) and deduped per (dst tile, src); E blocks are 0/1 multiplicity
    counts, exact in fp8, resident in SBUF, reused by both layers. Layer 2
    gathers the same slots from z1 (self slots read the core's own rows,
    already dinv-scaled, so no vector multiply).
  - Layer 1 processes buckets in (group: piece A then B) pair order,
    accumulating each 4-tile PSUM bank across both buckets. Layer 2
    processes all piece-A buckets first (so they only wait on AG_a),
    spilling each bank to an SBUF partial, then reopens from the partial
    during the piece-B phase.
  - Per dst tile close: agg[dst,feat] -> copy*dinv[dst] -> transpose ->
    w^T matmul -> bias(+relu) -> transpose -> store copy (*dinv for z1's
    pre-scale; plain f32 for the final output) -> one DMA per group ->
    piece AllGather dispatch after groups 5 / 12.
"""

import os

import numpy as np

import concourse.bacc as bacc
from bass_rust import InstructionNameOrderedSet
import concourse.mybir as mybir
import concourse.tile as tile
from concourse.bass_utils import run_bass_kernel_spmd
from concourse.library_config import mlp

# Problem shape (hardcoded per harness contract)
N = 50000
E = 600000
V = 50000
D_IN = 256
D = 128
NCORES = 8

NPC = N // NCORES            # 6250 nodes per core
TPC = (NPC + 127) // 128     # 49 tiles per core
NPAD = TPC * 128             # 6272 padded nodes per core
PA_T = 24                    # piece A: tiles 0..23
PB_T = TPC - PA_T            # piece B: tiles 24..48
RA = PA_T * 128              # 3072 rows per core in piece A
RB = PB_T * 128              # 3200 rows per core in piece B
VLO = V // 2                 # 25000: projected-table split
GRP = 4                      # dst tiles per aggregation group (PSUM bank)
NGRP = (TPC + GRP - 1) // GRP
GMAXC = 8                    # max chunks (x128 slots) per dma_gather
NQ = 4                       # SWDGE queues
F32 = mybir.dt.float32
BF16 = mybir.dt.bfloat16
FP8 = mybir.dt.float8e4
I16 = mybir.dt.int16
MSGB = int(os.environ.get("KMSGB", "12"))   # msgs pool bufs


def _wrap_idx(idx_linear):
    """[n] -> [128, n/16] int16: position j at [j%16, j//16], replicated x8."""
    n = idx_linear.shape[0]
    assert n % 16 == 0
    w = idx_linear.astype(np.int16).reshape(-1, 16).T
    return np.tile(w, (8, 1))


def _preprocess(node_tokens, edge_index):
    """Build per-core host arrays + the (core-uniform) schedules.

    Shared slot layout: buckets (piece p, group g), each split into two
    chunk-aligned sections by src-token half h. Within a section, slots
    sorted by (dst tile, src), deduped per (dst tile, src). Self edges
    (i -> i) are included for every padded node.

    Returns per-core {gidx1, gidx2, emat, dinvslot, dinv} plus layout:
      blocks1/blocks2: [(gidx_col_off, n_chunks, src_id, chunk_base)]
        src_id: layer 1 -> 0=tp_lo 1=tp_hi; layer 2 -> 0=z_fa 1=z_fb
      ops1/ops2: per linear chunk, ordered ops:
        ("bank", g) | ("reopen", g) | ("mm", t, eb, start, stop) |
        ("pclose", g) | ("close", t, g)
      n_eb, nch: E-block and chunk counts
    """
    import ml_dtypes

    src_e = np.asarray(edge_index[0], dtype=np.int64)
    dst_e = np.asarray(edge_index[1], dtype=np.int64)
    tok = np.asarray(node_tokens, dtype=np.int64)

    deg = (np.bincount(dst_e, minlength=N) + 1).astype(np.float32)
    dinv = (1.0 / np.sqrt(deg)).astype(np.float32)
    # per padded-node-id token / dinv (pads: tok 0, dinv 0)
    tokp = np.zeros(NCORES * NPAD, np.int64)
    dinvp = np.zeros(NCORES * NPAD, np.float32)
    for c in range(NCORES):
        tokp[c * NPAD : c * NPAD + NPC] = tok[c * NPC : (c + 1) * NPC]
        dinvp[c * NPAD : c * NPAD + NPC] = dinv[c * NPC : (c + 1) * NPC]

    # edge list in padded-node-id space + self loops for every padded node
    pid = lambda n: (n // NPC) * NPAD + (n % NPC)
    allid = np.arange(NCORES * NPAD)
    src = np.concatenate([pid(src_e), allid])
    dst = np.concatenate([pid(dst_e), allid])

    core = dst // NPAD
    dloc = dst % NPAD
    tloc = dloc // 128
    dcol = (dloc % 128).astype(np.int64)
    sloc = src % NPAD
    piece = (sloc >= RA).astype(np.int64)
    half = (tokp[src] >= VLO).astype(np.int64)
    grp = tloc // GRP

    # sort per (dst core, piece, group, half, dst tile, src)
    key = ((((core * 2 + piece) * NGRP + grp) * 2 + half) * TPC + tloc)
    order = np.lexsort((src, key))
    key_s = key[order]
    src_s = src[order]
    dcol_s = dcol[order]
    nkey = NCORES * 2 * NGRP * 2 * TPC
    counts_raw = np.bincount(key_s, minlength=nkey)
    starts = np.zeros(nkey + 1, dtype=np.int64)
    np.cumsum(counts_raw, out=starts[1:])

    def kid(c, p, g, h, t):
        return (((c * 2 + p) * NGRP + g) * 2 + h) * TPC + t

    groups = [list(range(g * GRP, min((g + 1) * GRP, TPC)))
              for g in range(NGRP)]

    dedup = {}
    counts = {}
    for c in range(NCORES):
        for p in (0, 1):
            for g, ts in enumerate(groups):
                for h in (0, 1):
                    for t in ts:
                        k = kid(c, p, g, h, t)
                        s0, ne = starts[k], int(counts_raw[k])
                        uidx, inv = np.unique(src_s[s0 : s0 + ne],
                                              return_inverse=True)
                        dedup[(c, p, g, h, t)] = (uidx, inv,
                                                  dcol_s[s0 : s0 + ne])
                        counts[(c, p, g, h, t)] = uidx.shape[0]

    # section chunk counts (maxed over cores) and per-core slot offsets
    nsec = np.zeros((2, NGRP, 2), np.int64)
    tstart = {}
    for p in (0, 1):
        for g, ts in enumerate(groups):
            for h in (0, 1):
                mx = 0
                for c in range(NCORES):
                    acc = 0
                    for t in ts:
                        tstart[(c, p, g, h, t)] = acc
                        acc += counts[(c, p, g, h, t)]
                    mx = max(mx, acc)
                nsec[p, g, h] = -(-mx // 128)
                assert nsec[p, g, h] >= 1

    # linear chunk ids: buckets (p, g) in p-major order; sections h0, h1
    bucket_base = {}
    sec_base = {}
    nch = 0
    for p in (0, 1):
        for g in range(NGRP):
            bucket_base[(p, g)] = nch
            sec_base[(p, g, 0)] = nch
            sec_base[(p, g, 1)] = nch + int(nsec[p, g, 0])
            nch += int(nsec[p, g, 0] + nsec[p, g, 1])

    # union chunk span (bucket-relative -> absolute) per (p, g, h, t)
    span = {}
    for p in (0, 1):
        for g, ts in enumerate(groups):
            for h in (0, 1):
                b = sec_base[(p, g, h)]
                for t in ts:
                    k0 = min(tstart[(c, p, g, h, t)] // 128
                             for c in range(NCORES))
                    k1 = max(-(-(tstart[(c, p, g, h, t)]
                                 + counts[(c, p, g, h, t)]) // 128)
                             for c in range(NCORES))
                    span[(p, g, h, t)] = (b + k0, b + max(k1, k0))

    # E-block ids in (p, g, h-section, chunk, tile) emission order
    eb_of = {}
    n_eb = 0
    bucket_mms = {}
    for p in (0, 1):
        for g, ts in enumerate(groups):
            mms = []
            for h in (0, 1):
                b = sec_base[(p, g, h)]
                for k in range(b, b + int(nsec[p, g, h])):
                    for t in ts:
                        k0, k1 = span[(p, g, h, t)]
                        if k0 <= k < k1:
                            eb_of[(k, t)] = n_eb
                            mms.append((k, t, n_eb))
                            n_eb += 1
            assert mms, f"empty bucket p={p} g={g}"
            bucket_mms[(p, g)] = mms
    ebarr = np.full((nch, GRP), -1, np.int64)
    for (k, t), eb in eb_of.items():
        ebarr[k, t % GRP] = eb

    def tile_close_chunk(g, t):
        """Chunk at which tile t closes (its last mm in bucket (1, g))."""
        cands = [k for (k, tt, _) in bucket_mms[(1, g)] if tt == t]
        return cands[-1] if cands else bucket_mms[(1, g)][-1][0]

    # ---- layer 1 ops: bucket pair order (0,g),(1,g); accumulate across ----
    ops1 = [[] for _ in range(nch)]
    for g in range(NGRP):
        mms0, mms1 = bucket_mms[(0, g)], bucket_mms[(1, g)]
        ops1[bucket_base[(0, g)]].append(("bank", g))
        for i, (k, t, eb) in enumerate(mms0):
            ops1[k].append(("mm", t, eb, i == 0, False))
        for i, (k, t, eb) in enumerate(mms1):
            ops1[k].append(("mm", t, eb, False, i == len(mms1) - 1))
        for t in groups[g]:
            ops1[tile_close_chunk(g, t)].append(("close", t, g))

    # ---- layer 2 ops: phase order; spill/reload SBUF partials ----
    ops2 = [[] for _ in range(nch)]
    for g in range(NGRP):
        mms0, mms1 = bucket_mms[(0, g)], bucket_mms[(1, g)]
        ops2[bucket_base[(0, g)]].append(("bank", g))
        for i, (k, t, eb) in enumerate(mms0):
            ops2[k].append(("mm", t, eb, i == 0, i == len(mms0) - 1))
        ops2[mms0[-1][0]].append(("pclose", g))
        ops2[bucket_base[(1, g)]].append(("reopen", g))
        for i, (k, t, eb) in enumerate(mms1):
            ops2[k].append(("mm", t, eb, False, i == len(mms1) - 1))
        for t in groups[g]:
            ops2[tile_close_chunk(g, t)].append(("close", t, g))

    kind_rank = {"bank": 0, "reopen": 0, "mm": 1, "pclose": 2, "close": 2}
    for ops in (ops1, ops2):
        for k in range(nch):
            ops[k].sort(key=lambda op: (kind_rank[op[0]],
                                        op[1] if op[0] == "close" else -1))

    # ---- gather blocks ----
    # layer 1: bucket PAIR order (0,g),(1,g) to match ops1's cross-bucket
    # bank accumulation; blocks per (p, g, h) section (source = Tp half h)
    blocks1 = []
    for g in range(NGRP):
        for p in (0, 1):
            for h in (0, 1):
                b = sec_base[(p, g, h)]
                nk = int(nsec[p, g, h])
                # small first blocks in the very first section: payload
                # starts after ~2us of desc-gen instead of ~9us
                cap = 2 if (g == 0 and p == 0 and h == 0) else GMAXC
                off = 0
                while off < nk:
                    n = min(cap, nk - off)
                    blocks1.append([None, n, h, b + off])
                    off += n
                    cap = min(cap * 2, GMAXC)
    # layer 2: per (p, g) bucket (source = z1 piece p)
    blocks2 = []
    for p in (0, 1):
        for g in range(NGRP):
            b = bucket_base[(p, g)]
            nk = int(nsec[p, g, 0] + nsec[p, g, 1])
            off = 0
            while off < nk:
                n = min(GMAXC, nk - off)
                blocks2.append([None, n, p, b + off])
                off += n
    col = 0
    for blk in blocks1:
        blk[0] = col
        col += blk[1] * 8
    g1cols = col
    col = 0
    for blk in blocks2:
        blk[0] = col
        col += blk[1] * 8
    g2cols = col

    per_core = []
    for c in range(NCORES):
        # linear slot tables (chunk-id space)
        lin1 = np.zeros(nch * 128, np.int64)         # Tp half-space index
        lin2 = np.zeros(nch * 128, np.int64)         # z1 piece-space index
        dslot = np.zeros(nch * 128, np.float32)      # dinv[src] per slot
        eflat = np.zeros((n_eb, 128, 128), np.float32)
        for p in (0, 1):
            for g, ts in enumerate(groups):
                for h in (0, 1):
                    b = sec_base[(p, g, h)]
                    for t in ts:
                        uidx, inv, dcols = dedup[(c, p, g, h, t)]
                        ne = uidx.shape[0]
                        if ne == 0:
                            continue
                        s0 = b * 128 + tstart[(c, p, g, h, t)]
                        lin1[s0 : s0 + ne] = tokp[uidx] - h * VLO
                        sl = uidx % NPAD
                        lin2[s0 : s0 + ne] = np.where(
                            sl < RA, (uidx // NPAD) * RA + sl,
                            (uidx // NPAD) * RB + (sl - RA))
                        dslot[s0 : s0 + ne] = dinvp[uidx]
                        gslots = (s0 - b * 128) + inv
                        ebs = ebarr[b + gslots // 128, t % GRP]
                        assert (ebs >= 0).all()
                        np.add.at(eflat, (ebs, gslots % 128, dcols), 1.0)
        gidx1 = np.concatenate(
            [_wrap_idx(lin1[b * 128 : (b + n) * 128])
             for (_, n, _, b) in blocks1], axis=1)
        gidx2 = np.concatenate(
            [_wrap_idx(lin2[b * 128 : (b + n) * 128])
             for (_, n, _, b) in blocks2], axis=1)
        dinvslot = np.ascontiguousarray(
            dslot.reshape(nch, 128).T)               # [128, nch]

        dv = dinvp[c * NPAD : (c + 1) * NPAD]
        dinv_loc = np.ascontiguousarray(dv.reshape(TPC, 128).T)

        emat = np.ascontiguousarray(
            eflat.transpose(1, 0, 2).reshape(128, n_eb * 128)
        ).astype(ml_dtypes.float8_e4m3)
        per_core.append({"gidx1": gidx1, "gidx2": gidx2, "emat": emat,
                         "dinvslot": dinvslot, "dinv": dinv_loc})

    layout = {"blocks1": blocks1, "blocks2": blocks2, "ops1": ops1,
              "ops2": ops2, "n_eb": n_eb, "nch": nch,
              "g1cols": g1cols, "g2cols": g2cols}
    return per_core, layout


def _build(layout):
    blocks1 = layout["blocks1"]
    blocks2 = layout["blocks2"]
    ops1 = layout["ops1"]
    ops2 = layout["ops2"]
    n_eb = layout["n_eb"]
    nch = layout["nch"]

    nc = bacc.Bacc("TRN2", target_bir_lowering=False, debug=False,
                   num_devices=NCORES, num_swdge_queues=NQ)

    tp_lo = nc.dram_tensor("tp_lo", [VLO, D], BF16, kind="ExternalInput")
    tp_hi = nc.dram_tensor("tp_hi", [V - VLO, D], BF16, kind="ExternalInput")
    g1_d = nc.dram_tensor("gidx1", [128, layout["g1cols"]], I16,
                          kind="ExternalInput")
    g2_d = nc.dram_tensor("gidx2", [128, layout["g2cols"]], I16,
                          kind="ExternalInput")
    emat_d = nc.dram_tensor("emat", [128, n_eb * 128], FP8,
                            kind="ExternalInput")
    dslot_d = nc.dram_tensor("dinvslot", [128, nch], F32,
                             kind="ExternalInput")
    dinv_d = nc.dram_tensor("dinv", [128, TPC], F32, kind="ExternalInput")
    w1t_d = nc.dram_tensor("w1t", [128, D], BF16, kind="ExternalInput")
    w2t_d = nc.dram_tensor("w2t", [128, D], BF16, kind="ExternalInput")
    bias_d = nc.dram_tensor("bias", [128, 2], F32, kind="ExternalInput")
    identb_d = nc.dram_tensor("identb", [128, 128], BF16, kind="ExternalInput")
    ident8_d = nc.dram_tensor("ident8", [128, 128], FP8, kind="ExternalInput")
    out_d = nc.dram_tensor("out", [NPAD, D], F32, kind="ExternalOutput")

    ACT = mybir.ActivationFunctionType

    with tile.TileContext(nc) as tc:
        with (
            tc.tile_pool(name="const", bufs=1) as cp,
            tc.tile_pool(name="msgs", bufs=MSGB) as msgp,
            tc.tile_pool(name="part", bufs=NGRP) as partp,
            tc.tile_pool(name="work", bufs=3) as wk,
            tc.tile_pool(name="stage", bufs=3) as stg,
            tc.tile_pool(name="psG", bufs=3, space="PSUM") as psG,
            tc.tile_pool(name="psT", bufs=2, space="PSUM") as psT,
            tc.tile_pool(name="psB", bufs=2, space="PSUM") as psB,
            tc.tile_pool(name="psC", bufs=1, space="PSUM") as psC,
            tc.tile_pool(name="dram", bufs=1, space="DRAM") as dram,
        ):
            nc.gpsimd.load_library(mlp)

            z1_a = dram.tile([RA, D], BF16)
            z1_b = dram.tile([RB, D], BF16)
            z1_fa = dram.tile([NCORES * RA, D], BF16, addr_space="Shared")
            z1_fb = dram.tile([NCORES * RB, D], BF16, addr_space="Shared")
            dum_l = dram.tile([16, D], BF16)
            dum_f = dram.tile([NCORES * 16, D], BF16, addr_space="Shared")

            def collective(z_loc, z_full):
                return nc.gpsimd.collective_compute(
                    "AllGather", mybir.AluOpType.bypass,
                    replica_groups=[list(range(NCORES))],
                    ins=[z_loc.opt()], outs=[z_full.opt()])

            # Warm up the collective fabric (first-collective barrier takes
            # 150-250us; run it concurrently with layer 1 from t=0).
            collective(dum_l, dum_f)

            g1_sb = cp.tile([128, layout["g1cols"]], I16)
            g2_sb = cp.tile([128, layout["g2cols"]], I16)
            emat_sb = cp.tile([128, n_eb, 128], FP8)
            dslot_sb = cp.tile([128, nch], F32)
            dinv_sb = cp.tile([128, TPC], F32)
            w1t_sb = cp.tile([128, D], BF16)
            w2t_sb = cp.tile([128, D], BF16)
            bias_sb = cp.tile([128, 2], F32)
            identb_sb = cp.tile([128, 128], BF16)
            ident8_sb = cp.tile([128, 128], FP8)
            gq = [0, layout["g1cols"] // 4, layout["g1cols"] // 2,
                  3 * layout["g1cols"] // 4, layout["g1cols"]]
            for qi in range(4):
                nc.sync.dma_start(g1_sb[:, gq[qi] : gq[qi + 1]],
                                  g1_d[:, gq[qi] : gq[qi + 1]])
            nc.sync.dma_start(dslot_sb[:], dslot_d[:])
            # E blocks in 4 slices so early matmuls start before the full
            # 12+MB load lands
            qs = [0, n_eb // 4, n_eb // 2, 3 * n_eb // 4, n_eb]
            for qi in range(4):
                a, b = qs[qi], qs[qi + 1]
                nc.sync.dma_start(
                    emat_sb[:, a:b, :],
                    emat_d[:, a * 128 : b * 128].rearrange(
                        "p (c f) -> p c f", f=128))
            nc.sync.dma_start(dinv_sb[:], dinv_d[:])
            nc.sync.dma_start(w1t_sb[:], w1t_d[:])
            nc.sync.dma_start(w2t_sb[:], w2t_d[:])
            nc.sync.dma_start(bias_sb[:], bias_d[:])
            nc.sync.dma_start(identb_sb[:], identb_d[:])
            nc.sync.dma_start(ident8_sb[:], ident8_d[:])
            nc.sync.dma_start(g2_sb[:], g2_d[:])

            qn = [0]

            def next_q():
                qn[0] = (qn[0] + 1) % NQ
                return qn[0]

            # Pre-touch the msgs buffers so pad slots never multiply
            # uninitialized SBUF into the PSUM accumulation.
            for _ in range(MSGB):
                mz = msgp.tile([128, GMAXC, D], BF16, name="m", tag="m")
                nc.vector.memset(mz[:], 0)

            def run_layer(blocks, ops, srcs, use_scale, wt_sb, bias_col,
                          relu, dest_a, dest_b, out_colls):
                open_ps = {}
                grp_ps = {}
                grp_part = {}
                grp_stage = {}
                grp_closed = {}

                def ntile_of(g):
                    return min(GRP, TPC - g * GRP)

                def op_bank(g, moving):
                    ntile = ntile_of(g)
                    ps = psG.tile([128, GRP * 128], F32, name="agg", tag="pG")
                    grp_ps[g] = ps
                    if moving is not None:
                        nc.tensor.matmul(
                            ps[:, 0 : ntile * 128], ident8_sb[:], moving,
                            start=True, stop=False, skip_group_check=True)
                    for j in range(ntile):
                        open_ps[g * GRP + j] = ps[:, j * 128 : (j + 1) * 128]
                    grp_stage[g] = stg.tile(
                        [128, ntile, D], F32 if dest_b is None else BF16,
                        name="stage1", tag="st1")
                    grp_closed[g] = 0

                def op_pclose(g):
                    ntile = ntile_of(g)
                    part = partp.tile([128, GRP, D], BF16, name="part",
                                      tag="part")
                    grp_part[g] = part
                    ps = grp_ps.pop(g)
                    nc.scalar.activation(
                        part[:, 0:ntile, :].rearrange("p t f -> p (t f)"),
                        ps[:, 0 : ntile * 128], ACT.Copy)
                    for j in range(ntile):
                        del open_ps[g * GRP + j]
                    del grp_stage[g], grp_closed[g]

                def op_reopen(g):
                    ntile = ntile_of(g)
                    part = grp_part.pop(g)
                    op_bank(g, part[:, 0:ntile, :]
                            .rearrange("p t f -> p (t f)"))

                def op_close(t, g):
                    ntile = ntile_of(g)
                    agg_sb = wk.tile([128, 128], BF16, name="agg_sb",
                                     tag="agg_sb")
                    nc.scalar.activation(agg_sb[:], open_ps.pop(t), ACT.Copy,
                                         scale=dinv_sb[:, t : t + 1])
                    aggT_ps = psT.tile([128, 128], BF16, name="aggT",
                                       tag="pT")
                    nc.tensor.matmul(aggT_ps[:], agg_sb[:], identb_sb[:],
                                     is_transpose=True, start=True, stop=True)
                    aggT_sb = wk.tile([128, 128], BF16, name="aggT_sb",
                                      tag="aggT_sb")
                    nc.scalar.activation(aggT_sb[:], aggT_ps[:], ACT.Copy)
                    yT_ps = psB.tile([128, 128], F32, name="yT", tag="pB")
                    nc.tensor.matmul(yT_ps[:], wt_sb[:], aggT_sb[:],
                                     start=True, stop=True)
                    yT_sb = wk.tile([128, 128], BF16, name="yT_sb",
                                    tag="yT_sb")
                    nc.scalar.activation(yT_sb[:], yT_ps[:],
                                         ACT.Relu if relu else ACT.Identity,
                                         bias=bias_col)
                    y_ps = psC.tile([128, 128], BF16, name="y", tag="pC")
                    nc.tensor.matmul(y_ps[:], yT_sb[:], identb_sb[:],
                                     is_transpose=True, start=True, stop=True)
                    gt0 = g * GRP
                    if dest_b is None:
                        nc.scalar.activation(grp_stage[g][:, t - gt0, :],
                                             y_ps[:], ACT.Copy)
                    else:
                        nc.scalar.activation(grp_stage[g][:, t - gt0, :],
                                             y_ps[:], ACT.Copy,
                                             scale=dinv_sb[:, t : t + 1])
                    grp_closed[g] += 1
                    if grp_closed[g] == ntile:
                        if dest_b is None:
                            dst_rows = dest_a[gt0 * 128
                                              : (gt0 + ntile) * 128, :]
                        elif gt0 < PA_T:
                            dst_rows = dest_a[gt0 * 128
                                              : (gt0 + ntile) * 128, :]
                        else:
                            dst_rows = dest_b[(gt0 - PA_T) * 128
                                              : (gt0 - PA_T + ntile) * 128, :]
                        nc.sync.dma_start(
                            dst_rows.rearrange("(t p) f -> p t f", p=128),
                            grp_stage[g][:])
                        del grp_ps[g], grp_stage[g], grp_closed[g]
                        if out_colls is not None:
                            if g == PA_T // GRP - 1:
                                out_colls[0]()
                            elif g == NGRP - 1:
                                out_colls[1]()

                gsb = g1_sb if use_scale else g2_sb
                for coloff, n, sid, base in blocks:
                    msgs = msgp.tile([128, GMAXC, D], BF16, name="m",
                                     tag="m")
                    nc.gpsimd.dma_gather(
                        msgs[:, 0:n, :], srcs[sid],
                        gsb[:, coloff : coloff + n * 8],
                        n * 128, n * 128, D, queue_num=next_q())
                    if use_scale:
                        # one broadcast multiply per gather block: slot ==
                        # partition, so dinv[src] is a per-partition scalar
                        # replicated along feats via a stride-0 AP
                        nc.vector.tensor_tensor(
                            msgs[:, 0:n, :], msgs[:, 0:n, :],
                            dslot_sb[:, base : base + n]
                            .to_broadcast((128, n, 128)),
                            mybir.AluOpType.mult)
                    for k in range(n):
                        ck = base + k
                        for op in ops[ck]:
                            if op[0] == "bank":
                                op_bank(op[1], None)
                            elif op[0] == "reopen":
                                op_reopen(op[1])
                            elif op[0] == "mm":
                                _, t, eb, st, sp = op
                                nc.tensor.matmul(
                                    open_ps[t], emat_sb[:, eb, :],
                                    msgs[:, k, :], start=st, stop=sp,
                                    skip_group_check=True)
                            elif op[0] == "pclose":
                                op_pclose(op[1])
                            elif op[0] == "close":
                                op_close(op[1], op[2])

            run_layer(blocks1, ops1, (tp_lo[:], tp_hi[:]), True, w1t_sb,
                      bias_sb[:, 0:1], True, z1_a, z1_b,
                      [lambda: collective(z1_a, z1_fa),
                       lambda: collective(z1_b, z1_fb)])
            run_layer(blocks2, ops2, (z1_fa[:], z1_fb[:]), False, w2t_sb,
                      bias_sb[:, 1:2], False, out_d.ap(), None, None)

    nc.compile()
    return nc


_CACHE = {}


def _run(inputs, trace=False):
    import ml_dtypes

    node_tokens = np.asarray(inputs["node_tokens"])
    edge_index = np.asarray(inputs["edge_index"])
    embed_table = np.asarray(inputs["embed_table"], dtype=np.float32)
    Wn = np.asarray(inputs["W_node_w"], dtype=np.float32)
    bn = np.asarray(inputs["W_node_b"], dtype=np.float32)
    w1 = np.asarray(inputs["w1"], dtype=np.float32)
    b1 = np.asarray(inputs["b1"], dtype=np.float32)
    w2 = np.asarray(inputs["w2"], dtype=np.float32)
    b2 = np.asarray(inputs["b2"], dtype=np.float32)

    per_core, layout = _preprocess(node_tokens, edge_index)

    if "nc" not in _CACHE:
        _CACHE["nc"] = _build(layout)
    nc = _CACHE["nc"]

    # Parameter-only preprocessing: fold the embedding projection.
    Tp = (embed_table @ Wn.T + bn).astype(ml_dtypes.bfloat16)   # [V, 128]
    tp_lo = Tp[:VLO]
    tp_hi = Tp[VLO:]
    bias = np.stack([b1, b2], axis=1).astype(np.float32)
    identb = np.eye(128, dtype=ml_dtypes.bfloat16)
    ident8 = np.eye(128, dtype=ml_dtypes.float8_e4m3)


    in_maps = []
    for c in range(NCORES):
        in_maps.append({
            "tp_lo": tp_lo, "tp_hi": tp_hi,
            "gidx1": per_core[c]["gidx1"],
            "gidx2": per_core[c]["gidx2"],
            "emat": per_core[c]["emat"],
            "dinvslot": per_core[c]["dinvslot"],
            "dinv": per_core[c]["dinv"],
            "w1t": w1.T.astype(ml_dtypes.bfloat16),
            "w2t": w2.T.astype(ml_dtypes.bfloat16),
            "bias": bias, "identb": identb, "ident8": ident8,
        })

    res = run_bass_kernel_spmd(nc, in_maps, core_ids=list(range(NCORES)),
                               trace=trace)
    out = np.concatenate([res.results[c]["out"][:NPC] for c in range(NCORES)],
                         axis=0)
    return out.astype(np.float32), res


def kernel(**inputs):
    out, _ = _run(inputs, trace=False)
    return out
